# revision 1
# baseline (speedup 1.0000x reference)
"""Trainium2 Bass kernel for 3-layer heterogeneous GraphSAGE (EntityGraphNN).

8 NeuronCores, SPMD single program, fp16 data path (f32 PSUM accumulate):
  - Destination-node sharding: each core owns 1/8 of each node type's
    128-row tiles. Edges routed to the core owning their dst.
  - h tables stored fp16 [N, 64]; gathers fetch PAIRS of rows (256B) via
    gpsimd dma_gather. Pair-index space is banked by 32768 (int16 limit);
    cells are (tile, pairbank, src-parity) so every 128-slot chunk is
    parity-pure and the matmul lhsT picks the right 64-wide half.
  - Scatter-add via one-hot matmul: oh[e,d] = (dstl[e]==d) built on DVE in
    BATCHED 3-D broadcast ops (one instr per <=16 chunks), fp16; PSUM
    agg_T[64, 512] accumulates G^T @ oh per 4-tile sub-stripe.
  - z = (agg@Wl)*invcnt + h_dst@Wr (+b); ReLU; store h fp16 row-major
    (shard -> AllGather table) and transposed (next layer's root term).
  - Input projections replicated on every core (no layer-0 AllGather).
  - AllGather per dst type right after its stripes finish; layer order
    c,r,j then r,j,c then c, so each AG overlaps the following compute.
    AG of cheval after layer 1 is never consumed -> skipped.
  - No global barriers: Tile's shadow-memory dep tracking orders the
    collectives against producers/consumers.
"""
import numpy as np

HID = 64
P = 128
PBANK = 32768  # pairs per gather bank (int16 index range)
N_CORES = 8
OVF_CAP = 1024  # max shared overflow slots per run before q escalation

_ETYPES = {
    "rev_part": ("c", "r"),
    "monte": ("c", "j"),
    "part": ("r", "c"),
    "rev_monte": ("j", "c"),
}
_DST_ETYPES = {"c": ["rev_part", "monte"], "r": ["part"], "j": ["rev_monte"]}
_STRIPE_T = {"c": 8, "r": 16, "j": 8}
_OH_BATCH = 16  # chunks per batched one-hot DVE op


def _ceil(a, b):
    return (a + b - 1) // b


def _prep_edge_type(ei, n_src_pad, tpc, q, T):
    """Pack edges into (stripe, pairbank) gather runs with parity-pure
    128-slot chunks. Returns None if any (run, parity) overflow exceeds
    OVF_CAP (caller escalates q)."""
    src = ei[0].astype(np.int64)
    dst = ei[1].astype(np.int64)
    npair = n_src_pad // 2
    nbp = _ceil(npair, PBANK)
    shard = tpc * P
    core = dst // shard
    tile = (dst % shard) // P
    pair = src >> 1
    par = src & 1
    pb = pair // PBANK

    stripes = []
    t0 = 0
    while t0 < tpc:
        stripes.append((t0, min(T, tpc - t0)))
        t0 += T

    order = np.lexsort((src, par, pb, tile, core))
    so, do_ = src[order], dst[order]
    key = ((core * tpc + tile) * nbp + pb) * 2 + par
    ko = key[order]
    ncell = N_CORES * tpc * nbp * 2
    cnts = np.bincount(key, minlength=ncell).reshape(N_CORES, tpc, nbp, 2)
    starts = np.zeros(ncell + 1, np.int64)
    np.cumsum(cnts.reshape(-1), out=starts[1:])

    cpt = q // P
    runs = []
    total_slots = 0
    total_dcols = 0
    for si, (ts, nt) in enumerate(stripes):
        for b in range(nbp):
            # shared (both-parity) overflow block, 128-padded
            oc_max = 0
            ovf_tiles = [set(), set()]
            for c in range(N_CORES):
                oc = 0
                for p2 in (0, 1):
                    ov = np.maximum(cnts[c, ts:ts + nt, b, p2] - q, 0)
                    oc += int(ov.sum())
                    for t_rel in np.nonzero(ov)[0]:
                        ovf_tiles[p2].add(int(t_rel))
                oc_max = max(oc_max, oc)
            if oc_max > OVF_CAP:
                return None
            novf = _ceil(oc_max, P) * P
            n_slots = 2 * nt * q + novf
            norm_map = [(p2, t_rel)
                        for p2 in (0, 1)
                        for t_rel in range(nt)
                        for _ in range(cpt)]
            runs.append(dict(si=si, b=b, ts=ts, nt=nt, n_slots=n_slots,
                             novf=novf,
                             ovf_tiles=(sorted(ovf_tiles[0]),
                                        sorted(ovf_tiles[1])),
                             norm_map=norm_map))
            total_slots += n_slots
            # dstl columns: 1 per normal chunk, 2 per shared overflow chunk
            total_dcols += len(norm_map) + 2 * (novf // P)

    w_tot = total_slots // 16
    idx_all = np.zeros((N_CORES, 16, w_tot), np.int16)
    dstl_all = np.full((N_CORES, P, total_dcols), -1.0, np.float16)

    wofs = 0
    cofs = 0
    for r in runs:
        ts, nt, b = r["ts"], r["nt"], r["b"]
        n_slots = r["n_slots"]
        novf = r["novf"]
        nnorm = len(r["norm_map"])
        for c in range(N_CORES):
            idxs = np.zeros(n_slots, np.int64)
            # normal-chunk labels + per-parity overflow labels
            dls = np.full((n_slots // P, P), -1.0, np.float32)
            ovf_lab = [np.full(novf, -1.0, np.float32) for _ in (0, 1)]
            base_ovf = 2 * nt * q
            ovf_off = 0
            for p2 in (0, 1):
                for t_rel in range(nt):
                    t = ts + t_rel
                    k = ((c * tpc + t) * nbp + b) * 2 + p2
                    s0, s1 = int(starts[k]), int(starts[k + 1])
                    cell_pair = (so[s0:s1] >> 1) - b * PBANK
                    cell_dst = do_[s0:s1] - (c * shard + t * P)
                    take = min(s1 - s0, q)
                    blk = (p2 * nt + t_rel) * q
                    idxs[blk:blk + take] = cell_pair[:take]
                    dls.reshape(-1)[blk:blk + take] = cell_dst[:take]
                    if s1 - s0 > take:
                        no = s1 - s0 - take
                        assert ovf_off + no <= novf
                        idxs[base_ovf + ovf_off:base_ovf + ovf_off + no] = \
                            cell_pair[take:]
                        ovf_lab[p2][ovf_off:ovf_off + no] = \
                            cell_dst[take:] + t_rel * P
                        ovf_off += no
            idx_all[c, :, wofs:wofs + n_slots // 16] = \
                idxs.reshape(n_slots // 16, 16).T
            # dstl layout: [normal chunks][ovf chunk p0, ovf chunk p1]...
            dcols = np.full((total_dcols_run(r), P), -1.0, np.float32)
            dcols[:nnorm] = dls[:nnorm]
            for k2 in range(novf // P):
                dcols[nnorm + 2 * k2] = ovf_lab[0][k2 * P:(k2 + 1) * P]
                dcols[nnorm + 2 * k2 + 1] = ovf_lab[1][k2 * P:(k2 + 1) * P]
            dstl_all[c, :, cofs:cofs + dcols.shape[0]] = \
                dcols.T.astype(np.float16)
        r["wofs"] = wofs
        r["cofs"] = cofs
        r["ch"] = n_slots // P
        r["dch"] = total_dcols_run(r)
        wofs += n_slots // 16
        cofs += r["dch"]

    idx_rep = np.tile(idx_all, (1, 8, 1))
    return dict(runs=runs, stripes=stripes, idx=idx_rep, dstl=dstl_all,
                nbp=nbp, q=q, cpt=cpt, n_chunks=total_dcols, w_tot=w_tot,
                max_ch=max(r["ch"] for r in runs),
                max_dch=max(r["dch"] for r in runs))


def total_dcols_run(r):
    return len(r["norm_map"]) + 2 * (r["novf"] // P)


def _prep_type(ei, n_src_pad, tpc, T):
    nbp = _ceil(n_src_pad // 2, PBANK)
    lam_cell = ei.shape[1] / (N_CORES * tpc) / (nbp * 2)
    q = max(P, (int(lam_cell) // P) * P)
    while True:
        r = _prep_edge_type(ei, n_src_pad, tpc, q, T)
        if r is not None:
            return r
        q += P


def _invcnt(ei, n_dst_pad, tpc):
    cnt = np.bincount(ei[1].astype(np.int64), minlength=n_dst_pad).astype(np.float32)
    inv = 1.0 / np.maximum(cnt, 1.0)
    return inv.reshape(N_CORES, tpc, P)


def kernel(**inputs):
    import concourse.bass as bass
    import concourse.mybir as mybir
    import concourse.tile as tile
    import concourse.bacc as bacc
    import jax
    from jax.sharding import Mesh, PartitionSpec, NamedSharding
    from jax.experimental.shard_map import shard_map
    from concourse.bass2jax import (_bass_exec_p, partition_id_tensor,
                                    install_neuronx_cc_hook)

    f32 = mybir.dt.float32
    f16 = mybir.dt.float16
    x_np = {"c": np.asarray(inputs["x_cheval"], np.float32),
            "j": np.asarray(inputs["x_jockey"], np.float32),
            "r": np.asarray(inputs["x_course"], np.float32)}
    NC = x_np["c"].shape[0]

    tpc = {k: _ceil(x_np[k].shape[0], P * N_CORES) for k in x_np}
    npad = {k: tpc[k] * P * N_CORES for k in tpc}
    din = {k: x_np[k].shape[1] + 1 for k in x_np}  # +1 ones row folds bias

    xT = {}
    for k in x_np:
        xt = np.zeros((din[k], npad[k]), np.float16)
        xt[:-1, :x_np[k].shape[0]] = x_np[k].T.astype(np.float16)
        xt[-1, :] = 1.0
        xT[k] = xt

    w_in_np = {}
    for k, nm in (("c", "cheval"), ("j", "jockey"), ("r", "course")):
        w = np.asarray(inputs[f"w_in_{nm}"], np.float32)
        b = np.asarray(inputs[f"b_in_{nm}"], np.float32)
        w_in_np[k] = np.concatenate([w, b.reshape(1, HID)], axis=0).astype(np.float16)

    w_cls = np.asarray(inputs["w_cls"], np.float32)
    b_cls = float(np.asarray(inputs["b_cls"]).reshape(-1)[0])
    eis = {k: np.asarray(inputs["ei_" + k]) for k in _ETYPES}
    NLAYERS = np.asarray(inputs["wl_part"]).shape[0]

    prep, iv = {}, {}
    for et, (dk, sk) in _ETYPES.items():
        prep[et] = _prep_type(eis[et], npad[sk], tpc[dk], _STRIPE_T[dk])
        iv[et] = _invcnt(eis[et], npad[dk], tpc[dk])

    WL = {et: np.asarray(inputs["wl_" + et], np.float16) for et in _ETYPES}
    BL = {et: np.asarray(inputs["bl_" + et], np.float32) for et in _ETYPES}
    WR = {et: np.asarray(inputs["wr_" + et], np.float32) for et in _ETYPES}
    WRc = {dk: sum(WR[et] for et in _DST_ETYPES[dk]).astype(np.float16)
           for dk in _DST_ETYPES}
    Bc = {dk: sum(BL[et] for et in _DST_ETYPES[dk]).astype(np.float16)
          for dk in _DST_ETYPES}
    bias_nonzero = {dk: bool(np.any(Bc[dk])) for dk in _DST_ETYPES}

    nc = bacc.Bacc(None, num_swdge_queues=4)

    din_t = {k: nc.declare_dram_parameter(f"xT_{k}", [din[k], npad[k]], f16, False)
             for k in tpc}
    xown_t = {k: nc.declare_dram_parameter(f"xo_{k}", [din[k], tpc[k] * P], f16, False)
              for k in tpc}
    win_t = {k: nc.declare_dram_parameter(f"win_{k}", [din[k], HID], f16, False)
             for k in tpc}
    idx_t = {et: nc.declare_dram_parameter(f"idx_{et}", [P, prep[et]["w_tot"]],
                                           mybir.dt.int16, False) for et in _ETYPES}
    dstl_t = {et: nc.declare_dram_parameter(f"dstl_{et}", [P, prep[et]["n_chunks"]],
                                            f16, False) for et in _ETYPES}
    ivc_t = {et: nc.declare_dram_parameter(f"ivc_{et}", [P, tpc[_ETYPES[et][0]]],
                                           f32, False) for et in _ETYPES}
    wl_t = {et: nc.declare_dram_parameter(f"wl_{et}", [NLAYERS, HID, HID], f16, False)
            for et in _ETYPES}
    wrc_t = {dk: nc.declare_dram_parameter(f"wrc_{dk}", [NLAYERS, HID, HID], f16,
                                           False) for dk in _DST_ETYPES}
    bc_t = {dk: nc.declare_dram_parameter(f"bc_{dk}", [NLAYERS, 1, HID], f16, False)
            for dk in _DST_ETYPES}
    iota128_t = nc.declare_dram_parameter("iota128", [P, P], f16, False)
    iotarep_t = nc.declare_dram_parameter("iotarep", [P, _OH_BATCH * P], f16, False)
    iotaT_t = {dk: nc.declare_dram_parameter(f"iotaT_{dk}", [P, _STRIPE_T[dk] * P],
                                             f16, False) for dk in _DST_ETYPES}
    wclsr_t = nc.declare_dram_parameter("wclsr", [P, _STRIPE_T["c"] * HID], f16, False)
    out_t = nc.declare_dram_parameter("out", [tpc["c"] * P, 1], f32, True)

    TC = _STRIPE_T["c"]

    with tile.TileContext(nc) as tc:
        with (
            tc.tile_pool(name="wpool", bufs=1) as wpool,
            tc.tile_pool(name="gpool", bufs=2) as gpool,
            tc.tile_pool(name="ohpool", bufs=2) as ohpool,
            tc.tile_pool(name="pool", bufs=2) as pool,
            tc.tile_pool(name="psum", bufs=2, space="PSUM") as psum,
        ):
            h_full = {}
            hT_loc = {}
            shard_buf = {}
            ag_out = {}
            for k in tpc:
                h0 = nc.dram_tensor(f"h0{k}", [npad[k], HID], f16)
                ag_out[k] = [nc.dram_tensor(f"ag{k}{l}", [npad[k], HID], f16,
                                            addr_space="Shared")
                             for l in range(2)]
                h_full[k] = [h0, ag_out[k][0], ag_out[k][1]]
                hT_loc[k] = [nc.dram_tensor(f"hT{k}{l}", [HID, tpc[k] * P], f16)
                             for l in range(3)]
                shard_buf[k] = [nc.dram_tensor(f"sh{k}{l}", [tpc[k] * P, HID], f16)
                                for l in range(2)]

            from concourse.masks import make_identity
            ident = wpool.tile([P, P], f16)
            make_identity(nc, ident[:])
            iota128 = wpool.tile([P, P], f16)
            nc.sync.dma_start(iota128[:], iota128_t[:])
            iotarep = wpool.tile([P, _OH_BATCH * P], f16)
            nc.sync.dma_start(iotarep[:], iotarep_t[:])
            iotaT = {}
            for dk in _DST_ETYPES:
                iotaT[dk] = wpool.tile([P, _STRIPE_T[dk] * P], f16,
                                       tag=f"iotaT{dk}", name=f"iotaT{dk}")
                nc.sync.dma_start(iotaT[dk][:], iotaT_t[dk][:])
            wclsr2 = wpool.tile([P, TC * HID], f16)
            nc.sync.dma_start(wclsr2[:], wclsr_t[:])
            ones1 = wpool.tile([1, P], f16)
            nc.gpsimd.memset(ones1[:], 1.0)
            win_sb = {}
            for k in tpc:
                win_sb[k] = wpool.tile([din[k], HID], f16, tag=f"win{k}", name=f"win{k}")
                nc.sync.dma_start(win_sb[k][:], win_t[k][:])
            wl_sb, wrc_sb, bc_sb = {}, {}, {}
            for et in _ETYPES:
                for l in range(NLAYERS):
                    wl_sb[(et, l)] = wpool.tile([HID, HID], f16, tag=f"wl{et}{l}",
                                                name=f"wl{et}{l}")
                    nc.sync.dma_start(wl_sb[(et, l)][:], wl_t[et][l])
            for dk in _DST_ETYPES:
                for l in range(NLAYERS):
                    wrc_sb[(dk, l)] = wpool.tile([HID, HID], f16, tag=f"wrc{dk}{l}",
                                                 name=f"wrc{dk}{l}")
                    nc.sync.dma_start(wrc_sb[(dk, l)][:], wrc_t[dk][l])
                    if bias_nonzero[dk]:
                        bc_sb[(dk, l)] = wpool.tile([1, HID], f16, tag=f"bc{dk}{l}",
                                                    name=f"bc{dk}{l}")
                        nc.sync.dma_start(bc_sb[(dk, l)][:], bc_t[dk][l])
            ivc_sb = {}
            for et in _ETYPES:
                ivc_sb[et] = wpool.tile([P, tpc[_ETYPES[et][0]]], f32,
                                        tag=f"ivc{et}", name=f"ivc{et}")
                nc.sync.dma_start(ivc_sb[et][:], ivc_t[et][:])

            # ---- input projection (full, replicated) ----
            GB = 4

            def proj_full(k):
                ntile = npad[k] // P
                for g0 in range(0, ntile, GB):
                    gn = min(GB, ntile - g0)
                    xt = gpool.tile([din[k], GB * P], f16, tag="xt")
                    nc.sync.dma_start(xt[:, :gn * P],
                                      din_t[k][:, g0 * P:(g0 + gn) * P])
                    zp = psum.tile([P, GB * HID], f32, space="PSUM", tag="zmr")
                    for j in range(gn):
                        nc.tensor.matmul(
                            out=zp[:, j * HID:(j + 1) * HID],
                            lhsT=xt[:, j * P:(j + 1) * P],
                            rhs=win_sb[k][:], start=(j == 0), stop=(j == gn - 1),
                            skip_group_check=True)
                    zr = pool.tile([P, GB * HID], f16, tag="zr")
                    nc.scalar.activation(
                        out=zr[:, :gn * HID], in_=zp[:, :gn * HID],
                        func=mybir.ActivationFunctionType.Relu)
                    oap = h_full[k][0][:][g0 * P:(g0 + gn) * P, :].rearrange(
                        "(t p) f -> p t f", p=P)
                    nc.sync.dma_start(oap, zr[:, :gn * HID].rearrange(
                        "p (t f) -> p t f", f=HID))

            def proj_own(k, GB=4):
                for g0 in range(0, tpc[k], GB):
                    gn = min(GB, tpc[k] - g0)
                    xt = gpool.tile([din[k], 8 * P], f16, tag="xt")
                    nc.sync.dma_start(xt[:, :gn * P],
                                      xown_t[k][:, g0 * P:(g0 + gn) * P])
                    ztp = psum.tile([HID, GB * P], f32, space="PSUM",
                                    tag=f"agg{(g0 // GB) % 2}", bufs=1)
                    for j in range(gn):
                        nc.tensor.matmul(
                            out=ztp[:, j * P:(j + 1) * P],
                            lhsT=win_sb[k][:],
                            rhs=xt[:, j * P:(j + 1) * P], start=(j == 0),
                            stop=(j == gn - 1), skip_group_check=True)
                    ztr = pool.tile([HID, GB * P], f16, tag="ztr")
                    nc.scalar.activation(
                        out=ztr[:, :gn * P], in_=ztp[:, :gn * P],
                        func=mybir.ActivationFunctionType.Relu)
                    nc.sync.dma_start(hT_loc[k][0][:, g0 * P:(g0 + gn) * P],
                                      ztr[:, :gn * P])

            # j/r tables first so layer-0 c-dst gathers can start while the
            # big cheval projection still runs
            for k in ("j", "r"):
                proj_full(k)
                proj_own(k)
            proj_own("c")

            # max gather-tile widths (chunks) per dst: tagA = primary et,
            # tagB = second et of dst c
            gta_ch = max(prep[et]["max_ch"]
                         for et in ("rev_part", "part", "rev_monte"))
            gtb_ch = prep["monte"]["max_ch"]

            qrot = [0]

            def do_stripe(dk, l, si, last):
                ets = _DST_ETYPES[dk]
                stripes = prep[ets[0]]["stripes"]
                ts, nt = stripes[si]
                nsub = _ceil(nt, 4)
                aggs = {}
                for eti, et in enumerate(ets):
                    pr = prep[et]
                    q = pr["q"]
                    cpt = pr["cpt"]
                    rlist = [r for r in pr["runs"] if r["si"] == si]
                    tag = "A" if eti == 0 else "B"
                    mx_ch = gta_ch if eti == 0 else gtb_ch
                    # per (sub) matmul counting for start/stop flags
                    mm_total = [0] * nsub
                    mm_seen = [0] * nsub
                    for r in rlist:
                        for (p2, t_rel) in r["norm_map"]:
                            mm_total[t_rel // 4] += 1
                        for _ in range(r["novf"] // P):
                            for p2 in (0, 1):
                                for t_rel in r["ovf_tiles"][p2]:
                                    mm_total[t_rel // 4] += 1
                    agg = []
                    for s in range(nsub):
                        atag = (f"agg{eti * 2 + s}" if nsub <= 2
                                else f"agg{s}")
                        agg.append(psum.tile([HID, 4 * P], f32, space="PSUM",
                                             tag=atag, name=atag, bufs=1))
                    aggs[et] = agg
                    for r in rlist:
                        ch = r["ch"]
                        n_slots = r["n_slots"]
                        wofs, cofs = r["wofs"], r["cofs"]
                        it = gpool.tile([P, mx_ch * P // 16],
                                        mybir.dt.int16, tag=f"it{tag}")
                        nc.sync.dma_start(it[:, :n_slots // 16],
                                          idx_t[et][:, wofs:wofs + n_slots // 16])
                        dch = r["dch"]
                        dt_ = gpool.tile([P, pr["max_dch"]], f16, tag=f"dt{tag}")
                        nc.sync.dma_start(dt_[:, :dch],
                                          dstl_t[et][:, cofs:cofs + dch])
                        gt = gpool.tile([P, mx_ch * P], f16, tag=f"gt{tag}")
                        sk = _ETYPES[et][1]
                        tab = h_full[sk][l]
                        b = r["b"]
                        pair_lo = b * PBANK
                        pair_hi = min((b + 1) * PBANK, npad[sk] // 2)
                        in_ap = tab[:][2 * pair_lo:2 * pair_hi, :].rearrange(
                            "(pr two) f -> pr (two f)", two=2)
                        nc.gpsimd.dma_gather(
                            out_ap=gt[:, :ch * P].rearrange(
                                "p (c f) -> p c f", f=P),
                            in_ap=in_ap,
                            idxs_ap=it[:, :n_slots // 16],
                            num_idxs=n_slots, num_idxs_reg=n_slots,
                            elem_size=P, single_packet=False,
                            queue_num=qrot[0] % 4)
                        qrot[0] += 1
                        # --- batched one-hot + matmuls: normal chunks ---
                        nnorm = len(r["norm_map"])
                        for c0 in range(0, nnorm, _OH_BATCH):
                            cn = min(_OH_BATCH, nnorm - c0)
                            oh = ohpool.tile([P, _OH_BATCH * P], f16,
                                             tag=f"oh{tag}")
                            nc.vector.tensor_tensor(
                                out=oh[:, :cn * P].rearrange(
                                    "p (c d) -> p c d", d=P),
                                in0=dt_[:, c0:c0 + cn].rearrange(
                                    "p (c u) -> p c u", u=1).to_broadcast(
                                    [P, cn, P]),
                                in1=iotarep[:, :cn * P].rearrange(
                                    "p (c d) -> p c d", d=P),
                                op=mybir.AluOpType.is_equal)
                            for cc in range(cn):
                                ci = c0 + cc
                                p2, t_rel = r["norm_map"][ci]
                                s = t_rel // 4
                                mm_seen[s] += 1
                                nc.tensor.matmul(
                                    out=agg[s][:, (t_rel % 4) * P:
                                               (t_rel % 4 + 1) * P],
                                    lhsT=gt[:, ci * P + p2 * HID:
                                            ci * P + p2 * HID + HID],
                                    rhs=oh[:, cc * P:(cc + 1) * P],
                                    start=(mm_seen[s] == 1),
                                    stop=(mm_seen[s] == mm_total[s]),
                                    skip_group_check=True)
                        # --- shared overflow chunks (labels span nt tiles,
                        # one gather chunk, two per-parity dstl columns) ---
                        for k2 in range(r["novf"] // P):
                            ci = nnorm + k2
                            for p2 in (0, 1):
                                if not r["ovf_tiles"][p2]:
                                    continue
                                dcol = nnorm + 2 * k2 + p2
                                oh = ohpool.tile([P, _STRIPE_T[dk] * P], f16,
                                                 tag=f"ohovf{tag}")
                                nc.vector.tensor_tensor(
                                    out=oh[:, :nt * P],
                                    in0=dt_[:, dcol:dcol + 1].to_broadcast(
                                        [P, nt * P]),
                                    in1=iotaT[dk][:, :nt * P],
                                    op=mybir.AluOpType.is_equal)
                                for t_rel in r["ovf_tiles"][p2]:
                                    s = t_rel // 4
                                    mm_seen[s] += 1
                                    nc.tensor.matmul(
                                        out=agg[s][:, (t_rel % 4) * P:
                                                   (t_rel % 4 + 1) * P],
                                        lhsT=gt[:, ci * P + p2 * HID:
                                                ci * P + p2 * HID + HID],
                                        rhs=oh[:, t_rel * P:(t_rel + 1) * P],
                                        start=(mm_seen[s] == 1),
                                        stop=(mm_seen[s] == mm_total[s]),
                                        skip_group_check=True)
                    assert mm_seen == mm_total, (dk, l, si, et, mm_seen, mm_total)
                # --- epilogue: aggs -> z -> relu -> stores ---
                aggsb = {}
                for et in ets:
                    a = pool.tile([HID, _STRIPE_T[dk] * P], f16, tag=f"aggsb{et}",
                                  name=f"aggsb{et}")
                    for s in range(nsub):
                        w = min(4 * P, (nt - s * 4) * P)
                        nc.scalar.activation(
                            out=a[:, s * 4 * P:s * 4 * P + w],
                            in_=aggs[et][s][:, :w],
                            func=mybir.ActivationFunctionType.Copy)
                    aggsb[et] = a
                hTs = pool.tile([HID, _STRIPE_T[dk] * P], f16, tag="hTs")
                nc.sync.dma_start(hTs[:, :nt * P],
                                  hT_loc[dk][l][:][:, ts * P:(ts + nt) * P])
                zsb = pool.tile([P, _STRIPE_T[dk] * HID], f16, tag="zsb")
                for j in range(nt):
                    t = ts + j
                    zmr = psum.tile([P, 4 * HID], f32, space="PSUM", tag="zmr")
                    for ei_, et in enumerate(ets):
                        nc.tensor.matmul(
                            out=zmr[:, ei_ * HID:(ei_ + 1) * HID],
                            lhsT=aggsb[et][:, j * P:(j + 1) * P],
                            rhs=wl_sb[(et, l)][:],
                            start=(ei_ == 0), stop=False,
                            skip_group_check=True)
                    ro = 2 * HID
                    nc.tensor.matmul(out=zmr[:, ro:ro + HID],
                                     lhsT=hTs[:, j * P:(j + 1) * P],
                                     rhs=wrc_sb[(dk, l)][:],
                                     start=False,
                                     stop=not bias_nonzero[dk],
                                     skip_group_check=True)
                    if bias_nonzero[dk]:
                        nc.tensor.matmul(out=zmr[:, ro:ro + HID],
                                         lhsT=ones1[:],
                                         rhs=bc_sb[(dk, l)][:],
                                         start=False, stop=True,
                                         skip_group_check=True)
                    # z = sum_et ivc_et * zm_et + zroot, then relu (fp16 out)
                    zrt = pool.tile([P, HID], f32, tag="zrt")
                    nc.scalar.activation(
                        out=zrt[:], in_=zmr[:, ro:ro + HID],
                        func=mybir.ActivationFunctionType.Copy)
                    tmp = pool.tile([P, HID], f32, tag="ztmp")
                    nc.vector.scalar_tensor_tensor(
                        out=tmp[:],
                        in0=zmr[:, 0:HID],
                        scalar=ivc_sb[ets[0]][:, t:t + 1],
                        in1=zrt[:],
                        op0=mybir.AluOpType.mult,
                        op1=mybir.AluOpType.add)
                    if len(ets) > 1:
                        nc.vector.scalar_tensor_tensor(
                            out=tmp[:],
                            in0=zmr[:, HID:2 * HID],
                            scalar=ivc_sb[ets[1]][:, t:t + 1],
                            in1=tmp[:],
                            op0=mybir.AluOpType.mult,
                            op1=mybir.AluOpType.add)
                    nc.scalar.activation(
                        out=zsb[:, j * HID:(j + 1) * HID], in_=tmp[:],
                        func=mybir.ActivationFunctionType.Relu)
                    if not last:
                        # transposed copy for next layer's root term
                        ztp = psum.tile([HID, P], f16, space="PSUM", tag="ztp")
                        nc.tensor.transpose(
                            out=ztp[:, :],
                            in_=zsb[:, j * HID:(j + 1) * HID],
                            identity=ident[:])
                        ztr2 = pool.tile([HID, P], f16, tag="ztr2")
                        nc.scalar.activation(
                            out=ztr2[:], in_=ztp[:, :],
                            func=mybir.ActivationFunctionType.Copy)
                        nc.sync.dma_start(
                            hT_loc[dk][l + 1][:][:, t * P:(t + 1) * P],
                            ztr2[:])
                if not last:
                    if not (dk == "c" and l == NLAYERS - 2):
                        oap = shard_buf[dk][l][:][ts * P:(ts + nt) * P, :].rearrange(
                            "(t p) f -> p t f", p=P)
                        nc.sync.dma_start(oap, zsb[:, :nt * HID].rearrange(
                            "p (t f) -> p t f", f=HID))
                else:
                    tmp2 = pool.tile([P, TC * HID], f16, tag="ctmp")
                    nc.vector.tensor_tensor(
                        out=tmp2[:, :nt * HID], in0=zsb[:, :nt * HID],
                        in1=wclsr2[:, :nt * HID],
                        op=mybir.AluOpType.mult)
                    ot = pool.tile([P, TC], f32, tag="otile")
                    nc.vector.tensor_reduce(
                        out=ot[:, :nt],
                        in_=tmp2[:, :nt * HID].rearrange(
                            "p (t f) -> p t f", f=HID),
                        axis=mybir.AxisListType.X,
                        op=mybir.AluOpType.add)
                    if b_cls != 0.0:
                        nc.vector.tensor_scalar(
                            out=ot[:, :nt], in0=ot[:, :nt],
                            scalar1=b_cls, scalar2=None,
                            op0=mybir.AluOpType.add)
                    oap = out_t[:].rearrange("(t p) o -> p t o", p=P)
                    nc.sync.dma_start(oap[:, ts:ts + nt, 0], ot[:, :nt])

            def do_dst(dk, l):
                last = (l == NLAYERS - 1)
                stripes = prep[_DST_ETYPES[dk][0]]["stripes"]
                for si in range(len(stripes)):
                    do_stripe(dk, l, si, last)
                if not last and not (dk == "c" and l == NLAYERS - 2):
                    nc.gpsimd.collective_compute(
                        "AllGather", mybir.AluOpType.bypass,
                        ins=[shard_buf[dk][l][:]],
                        outs=[ag_out[dk][l][:]],
                        replica_groups=[list(range(N_CORES))])

            order = {0: ["c", "r", "j"], 1: ["r", "j", "c"], 2: ["c"]}
            for l in range(NLAYERS):
                for dk in order[min(l, 2)] if NLAYERS == 3 else ["c", "r", "j"]:
                    if l == NLAYERS - 1 and dk != "c":
                        continue
                    do_dst(dk, l)
                    if l == 0 and dk == "c":
                        proj_full("c")  # overlaps c-dst layer-0 gathers

    nc.finalize()

    iota128_v = np.broadcast_to(np.arange(P, dtype=np.float16), (P, P)).copy()
    iotarep_v = np.broadcast_to(
        np.tile(np.arange(P, dtype=np.float16), _OH_BATCH),
        (P, _OH_BATCH * P)).copy()
    iotaT_v = {dk: np.broadcast_to(
        np.arange(_STRIPE_T[dk] * P, dtype=np.float16),
        (P, _STRIPE_T[dk] * P)).copy() for dk in _DST_ETYPES}
    wclsr_v = np.tile(w_cls.reshape(1, HID), (P, TC)).astype(np.float16)

    in_maps = []
    for c in range(N_CORES):
        m = {}
        for k in tpc:
            sh = tpc[k] * P
            m[f"xT_{k}"] = xT[k]
            m[f"xo_{k}"] = np.ascontiguousarray(xT[k][:, c * sh:(c + 1) * sh])
            m[f"win_{k}"] = w_in_np[k]
        for et in _ETYPES:
            m[f"idx_{et}"] = prep[et]["idx"][c]
            m[f"dstl_{et}"] = prep[et]["dstl"][c]
            m[f"ivc_{et}"] = np.ascontiguousarray(iv[et][c].T)
            m[f"wl_{et}"] = WL[et]
        for dk in _DST_ETYPES:
            m[f"wrc_{dk}"] = WRc[dk]
            m[f"bc_{dk}"] = Bc[dk].reshape(NLAYERS, 1, HID)
            m[f"iotaT_{dk}"] = iotaT_v[dk]
        m["iota128"] = iota128_v
        m["iotarep"] = iotarep_v
        m["wclsr"] = wclsr_v
        in_maps.append(m)

    install_neuronx_cc_hook()
    partition_name = nc.partition_id_tensor.name if nc.partition_id_tensor else None
    in_names, out_names, out_avals, zero_outs = [], [], [], []
    for alloc in nc.m.functions[0].allocations:
        if not isinstance(alloc, mybir.MemoryLocationSet):
            continue
        name = alloc.memorylocations[0].name
        if alloc.kind == "ExternalInput":
            if name != partition_name:
                in_names.append(name)
        elif alloc.kind == "ExternalOutput":
            out_names.append(name)
            shape = tuple(alloc.tensor_shape)
            dtype = mybir.dt.np(alloc.dtype)
            out_avals.append(jax.core.ShapedArray(shape, dtype))
            zero_outs.append(np.zeros(shape, dtype))
    n_params = len(in_names)
    all_in = list(in_names) + list(out_names)
    if partition_name is not None:
        all_in.append(partition_name)

    def _body(*args):
        operands = list(args)
        if partition_name is not None:
            operands.append(partition_id_tensor())
        outs = _bass_exec_p.bind(
            *operands, out_avals=tuple(out_avals), in_names=tuple(all_in),
            out_names=tuple(out_names), lowering_input_output_aliases=(),
            sim_require_finite=False, sim_require_nnan=False, nc=nc)
        return tuple(outs)

    devices = jax.devices()[:N_CORES]
    mesh = Mesh(np.asarray(devices), ("core",))
    specs = (PartitionSpec("core"),)
    sharded = jax.jit(
        shard_map(_body, mesh=mesh, in_specs=specs * (n_params + len(out_names)),
                  out_specs=specs * len(out_names), check_rep=False),
        keep_unused=True)
    per_core = [[np.asarray(m[n]) for n in in_names] for m in in_maps]
    concat_in = [np.concatenate([per_core[c][i] for c in range(N_CORES)], axis=0)
                 for i in range(n_params)]
    concat_zero = [np.zeros((N_CORES * z.shape[0], *z.shape[1:]), z.dtype)
                   for z in zero_outs]
    shd = NamedSharding(mesh, PartitionSpec("core"))
    dev_in = [
        jax.make_array_from_callback(a.shape, shd, lambda idx, a=a: a[idx])
        for a in concat_in + concat_zero
    ]
    outs = sharded(*dev_in)
    jax.block_until_ready(outs)
    global _PROF
    _PROF = dict(sharded=sharded, dev_in=dev_in, nc=nc, out_names=out_names)
    oi = out_names.index("out")
    full = np.asarray(outs[oi]).reshape(N_CORES * tpc["c"] * P, 1)
    return full[:NC, :].astype(np.float32)



# revision 20
# speedup vs baseline: 1.0977x; 1.0977x over previous
"""Trainium2 Bass kernel for 3-layer heterogeneous GraphSAGE (EntityGraphNN).

8 NeuronCores, SPMD single program, fp16 data path (f32 PSUM accumulate):
  - Destination-node sharding: each core owns 1/8 of each node type's
    128-row tiles. Edges routed to the core owning their dst.
  - h tables stored fp16 [N, 64]; gathers fetch PAIRS of rows (256B) via
    gpsimd dma_gather. Pair-index space is banked by 32768 (int16 limit);
    cells are (tile, pairbank, src-parity) so every 128-slot chunk is
    parity-pure and the matmul lhsT picks the right 64-wide half.
  - Scatter-add via one-hot matmul: oh[e,d] = (dstl[e]==d) built on DVE in
    BATCHED 3-D broadcast ops (one instr per <=16 chunks), fp16; PSUM
    agg_T[64, 512] accumulates G^T @ oh per 4-tile sub-stripe.
  - z = (agg@Wl)*invcnt + h_dst@Wr (+b); ReLU; store h fp16 row-major
    (shard -> AllGather table) and transposed (next layer's root term).
  - Input projections replicated on every core (no layer-0 AllGather).
  - AllGather per dst type right after its stripes finish; layer order
    c,r,j then r,j,c then c, so each AG overlaps the following compute.
    AG of cheval after layer 1 is never consumed -> skipped.
  - No global barriers: Tile's shadow-memory dep tracking orders the
    collectives against producers/consumers.
"""
import numpy as np

HID = 64
P = 128
PBANK = 32768  # pairs per gather bank (int16 index range)
N_CORES = 8
OVF_CAP = 1024  # max shared overflow slots per run before q escalation

_ETYPES = {
    "rev_part": ("c", "r"),
    "monte": ("c", "j"),
    "part": ("r", "c"),
    "rev_monte": ("j", "c"),
}
_DST_ETYPES = {"c": ["rev_part", "monte"], "r": ["part"], "j": ["rev_monte"]}
_STRIPE_T = {"c": 8, "r": 16, "j": 8}
_OH_BATCH = 16  # chunks per batched one-hot DVE op


def _ceil(a, b):
    return (a + b - 1) // b


def _prep_edge_type(ei, n_src_pad, tpc, q, T):
    """Pack edges into (stripe, pairbank) gather runs with parity-pure
    128-slot chunks. Returns None if any (run, parity) overflow exceeds
    OVF_CAP (caller escalates q)."""
    src = ei[0].astype(np.int64)
    dst = ei[1].astype(np.int64)
    npair = n_src_pad // 2
    nbp = _ceil(npair, PBANK)
    shard = tpc * P
    core = dst // shard
    tile = (dst % shard) // P
    pair = src >> 1
    par = src & 1
    pb = pair // PBANK

    stripes = []
    t0 = 0
    while t0 < tpc:
        stripes.append((t0, min(T, tpc - t0)))
        t0 += T

    order = np.lexsort((src, par, pb, tile, core))
    so, do_ = src[order], dst[order]
    key = ((core * tpc + tile) * nbp + pb) * 2 + par
    ko = key[order]
    ncell = N_CORES * tpc * nbp * 2
    cnts = np.bincount(key, minlength=ncell).reshape(N_CORES, tpc, nbp, 2)
    starts = np.zeros(ncell + 1, np.int64)
    np.cumsum(cnts.reshape(-1), out=starts[1:])

    cpt = q // P
    runs = []
    total_slots = 0
    total_dcols = 0
    for si, (ts, nt) in enumerate(stripes):
        for b in range(nbp):
            # shared (both-parity) overflow block, 128-padded
            oc_max = 0
            ovf_tiles = [set(), set()]
            for c in range(N_CORES):
                oc = 0
                for p2 in (0, 1):
                    ov = np.maximum(cnts[c, ts:ts + nt, b, p2] - q, 0)
                    oc += int(ov.sum())
                    for t_rel in np.nonzero(ov)[0]:
                        ovf_tiles[p2].add(int(t_rel))
                oc_max = max(oc_max, oc)
            if oc_max > OVF_CAP:
                return None
            novf = _ceil(oc_max, P) * P
            n_slots = 2 * nt * q + novf
            norm_map = [(p2, t_rel)
                        for p2 in (0, 1)
                        for t_rel in range(nt)
                        for _ in range(cpt)]
            runs.append(dict(si=si, b=b, ts=ts, nt=nt, n_slots=n_slots,
                             novf=novf,
                             ovf_tiles=(sorted(ovf_tiles[0]),
                                        sorted(ovf_tiles[1])),
                             norm_map=norm_map))
            total_slots += n_slots
            # dstl columns: 1 per normal chunk, 2 per shared overflow chunk
            total_dcols += len(norm_map) + 2 * (novf // P)

    w_tot = total_slots // 16
    idx_all = np.zeros((N_CORES, 16, w_tot), np.int16)
    dstl_all = np.full((N_CORES, P, total_dcols), -1.0, np.float16)

    wofs = 0
    cofs = 0
    for r in runs:
        ts, nt, b = r["ts"], r["nt"], r["b"]
        n_slots = r["n_slots"]
        novf = r["novf"]
        nnorm = len(r["norm_map"])
        for c in range(N_CORES):
            idxs = np.zeros(n_slots, np.int64)
            # normal-chunk labels + per-parity overflow labels
            dls = np.full((n_slots // P, P), -1.0, np.float32)
            ovf_lab = [np.full(novf, -1.0, np.float32) for _ in (0, 1)]
            base_ovf = 2 * nt * q
            ovf_off = 0
            for p2 in (0, 1):
                for t_rel in range(nt):
                    t = ts + t_rel
                    k = ((c * tpc + t) * nbp + b) * 2 + p2
                    s0, s1 = int(starts[k]), int(starts[k + 1])
                    cell_pair = (so[s0:s1] >> 1) - b * PBANK
                    cell_dst = do_[s0:s1] - (c * shard + t * P)
                    take = min(s1 - s0, q)
                    blk = (p2 * nt + t_rel) * q
                    idxs[blk:blk + take] = cell_pair[:take]
                    dls.reshape(-1)[blk:blk + take] = cell_dst[:take]
                    if s1 - s0 > take:
                        no = s1 - s0 - take
                        assert ovf_off + no <= novf
                        idxs[base_ovf + ovf_off:base_ovf + ovf_off + no] = \
                            cell_pair[take:]
                        ovf_lab[p2][ovf_off:ovf_off + no] = \
                            cell_dst[take:] + t_rel * P
                        ovf_off += no
            idx_all[c, :, wofs:wofs + n_slots // 16] = \
                idxs.reshape(n_slots // 16, 16).T
            # dstl layout: [normal chunks][ovf chunk p0, ovf chunk p1]...
            dcols = np.full((total_dcols_run(r), P), -1.0, np.float32)
            dcols[:nnorm] = dls[:nnorm]
            for k2 in range(novf // P):
                dcols[nnorm + 2 * k2] = ovf_lab[0][k2 * P:(k2 + 1) * P]
                dcols[nnorm + 2 * k2 + 1] = ovf_lab[1][k2 * P:(k2 + 1) * P]
            dstl_all[c, :, cofs:cofs + dcols.shape[0]] = \
                dcols.T.astype(np.float16)
        r["wofs"] = wofs
        r["cofs"] = cofs
        r["ch"] = n_slots // P
        r["dch"] = total_dcols_run(r)
        wofs += n_slots // 16
        cofs += r["dch"]

    idx_rep = np.tile(idx_all, (1, 8, 1))
    return dict(runs=runs, stripes=stripes, idx=idx_rep, dstl=dstl_all,
                nbp=nbp, q=q, cpt=cpt, n_chunks=total_dcols, w_tot=w_tot,
                max_ch=max(r["ch"] for r in runs),
                max_dch=max(r["dch"] for r in runs))


def total_dcols_run(r):
    return len(r["norm_map"]) + 2 * (r["novf"] // P)


def _prep_type(ei, n_src_pad, tpc, T):
    nbp = _ceil(n_src_pad // 2, PBANK)
    lam_cell = ei.shape[1] / (N_CORES * tpc) / (nbp * 2)
    q = max(P, (int(lam_cell) // P) * P)
    while True:
        r = _prep_edge_type(ei, n_src_pad, tpc, q, T)
        if r is not None:
            return r
        q += P


def _invcnt(ei, n_dst_pad, tpc):
    cnt = np.bincount(ei[1].astype(np.int64), minlength=n_dst_pad).astype(np.float32)
    inv = 1.0 / np.maximum(cnt, 1.0)
    return inv.reshape(N_CORES, tpc, P)


def kernel(**inputs):
    import concourse.bass as bass
    import concourse.mybir as mybir
    import concourse.tile as tile
    import concourse.bacc as bacc
    import jax
    from jax.sharding import Mesh, PartitionSpec, NamedSharding
    from jax.experimental.shard_map import shard_map
    from concourse.bass2jax import (_bass_exec_p, partition_id_tensor,
                                    install_neuronx_cc_hook)

    f32 = mybir.dt.float32
    f16 = mybir.dt.float16
    x_np = {"c": np.asarray(inputs["x_cheval"], np.float32),
            "j": np.asarray(inputs["x_jockey"], np.float32),
            "r": np.asarray(inputs["x_course"], np.float32)}
    NC = x_np["c"].shape[0]

    tpc = {k: _ceil(x_np[k].shape[0], P * N_CORES) for k in x_np}
    npad = {k: tpc[k] * P * N_CORES for k in tpc}
    din = {k: x_np[k].shape[1] + 1 for k in x_np}  # +1 ones row folds bias

    xT = {}
    for k in x_np:
        xt = np.zeros((din[k], npad[k]), np.float16)
        xt[:-1, :x_np[k].shape[0]] = x_np[k].T.astype(np.float16)
        xt[-1, :] = 1.0
        xT[k] = xt

    w_in_np = {}
    for k, nm in (("c", "cheval"), ("j", "jockey"), ("r", "course")):
        w = np.asarray(inputs[f"w_in_{nm}"], np.float32)
        b = np.asarray(inputs[f"b_in_{nm}"], np.float32)
        w_in_np[k] = np.concatenate([w, b.reshape(1, HID)], axis=0).astype(np.float16)

    w_cls = np.asarray(inputs["w_cls"], np.float32)
    b_cls = float(np.asarray(inputs["b_cls"]).reshape(-1)[0])
    eis = {k: np.asarray(inputs["ei_" + k]) for k in _ETYPES}
    NLAYERS = np.asarray(inputs["wl_part"]).shape[0]

    # ---- piece-major table layout -------------------------------------
    # AllGathers are chunked; the BIR verifier requires contiguous
    # collective outputs, so tables are stored piece-major: for each row
    # piece [r0:r1) (aligned to stripe boundaries of the producing dst
    # phase), all 8 cores' rows are contiguous. Gather indices are
    # remapped host-side; dst-side structures stay in natural order.
    _N_PIECES = {"c": 4, "r": 2, "j": 1}

    def _mk_stripes(tpc_k, T):
        out = []
        t0 = 0
        while t0 < tpc_k:
            out.append((t0, min(T, tpc_k - t0)))
            t0 += T
        return out

    pieces = {}
    piecemap = {}
    for k in tpc:
        stripes_k = _mk_stripes(tpc[k], _STRIPE_T[k])
        nstr = len(stripes_k)
        npc = min(_N_PIECES[k], nstr)
        bounds = []
        r0 = 0
        for j in range(npc):
            es = ((j + 1) * nstr) // npc
            r1 = (stripes_k[es - 1][0] + stripes_k[es - 1][1]) * P
            if r1 > r0:
                bounds.append((r0, r1, es))
            r0 = r1
        pieces[k] = bounds
        S = tpc[k] * P
        pm = np.empty(npad[k], np.int64)
        base = 0
        for (r0, r1, _es) in bounds:
            sz = r1 - r0
            for c in range(N_CORES):
                pm[c * S + r0:c * S + r1] = base + c * sz + np.arange(sz)
            base += N_CORES * sz
        piecemap[k] = pm

    prep, iv = {}, {}
    for et, (dk, sk) in _ETYPES.items():
        ei_remap = np.stack([piecemap[sk][eis[et][0].astype(np.int64)],
                             eis[et][1].astype(np.int64)])
        prep[et] = _prep_type(ei_remap, npad[sk], tpc[dk], _STRIPE_T[dk])
        iv[et] = _invcnt(eis[et], npad[dk], tpc[dk])

    WL = {et: np.asarray(inputs["wl_" + et], np.float16) for et in _ETYPES}
    BL = {et: np.asarray(inputs["bl_" + et], np.float32) for et in _ETYPES}
    WR = {et: np.asarray(inputs["wr_" + et], np.float32) for et in _ETYPES}
    WRc = {dk: sum(WR[et] for et in _DST_ETYPES[dk]).astype(np.float16)
           for dk in _DST_ETYPES}
    Bc = {dk: sum(BL[et] for et in _DST_ETYPES[dk]).astype(np.float16)
          for dk in _DST_ETYPES}
    bias_nonzero = {dk: bool(np.any(Bc[dk])) for dk in _DST_ETYPES}

    nc = bacc.Bacc(None, num_swdge_queues=4, dynamic_dma_scratch_size=49152)

    din_t = {k: nc.declare_dram_parameter(f"xT_{k}", [din[k], npad[k]], f16, False)
             for k in tpc}
    xown_t = {k: nc.declare_dram_parameter(f"xo_{k}", [din[k], tpc[k] * P], f16, False)
              for k in tpc}
    win_t = {k: nc.declare_dram_parameter(f"win_{k}", [din[k], HID], f16, False)
             for k in tpc}
    idx_t = {et: nc.declare_dram_parameter(f"idx_{et}", [P, prep[et]["w_tot"]],
                                           mybir.dt.int16, False) for et in _ETYPES}
    dstl_t = {et: nc.declare_dram_parameter(f"dstl_{et}", [P, prep[et]["n_chunks"]],
                                            f16, False) for et in _ETYPES}
    ivc_t = {et: nc.declare_dram_parameter(f"ivc_{et}", [P, tpc[_ETYPES[et][0]]],
                                           f32, False) for et in _ETYPES}
    wl_t = {et: nc.declare_dram_parameter(f"wl_{et}", [NLAYERS, HID, HID], f16, False)
            for et in _ETYPES}
    wrc_t = {dk: nc.declare_dram_parameter(f"wrc_{dk}", [NLAYERS, HID, HID], f16,
                                           False) for dk in _DST_ETYPES}
    bc_t = {dk: nc.declare_dram_parameter(f"bc_{dk}", [NLAYERS, 1, HID], f16, False)
            for dk in _DST_ETYPES}
    iota128_t = nc.declare_dram_parameter("iota128", [P, P], f16, False)
    iotarep_t = nc.declare_dram_parameter("iotarep", [P, _OH_BATCH * P], f16, False)
    iotaT_t = {dk: nc.declare_dram_parameter(f"iotaT_{dk}", [P, _STRIPE_T[dk] * P],
                                             f16, False) for dk in _DST_ETYPES}
    wclsr_t = nc.declare_dram_parameter("wclsr", [P, _STRIPE_T["c"] * HID], f16, False)
    out_t = nc.declare_dram_parameter("out", [tpc["c"] * P, 1], f32, True)

    TC = _STRIPE_T["c"]

    with tile.TileContext(nc) as tc:
        with (
            tc.tile_pool(name="wpool", bufs=1) as wpool,
            tc.tile_pool(name="gpool", bufs=3) as gpool,
            tc.tile_pool(name="ohpool", bufs=2) as ohpool,
            tc.tile_pool(name="pool", bufs=2) as pool,
            tc.tile_pool(name="psum", bufs=2, space="PSUM") as psum,
        ):
            h_full = {}
            hT_loc = {}
            shard_buf = {}
            ag_out = {}
            for k in tpc:
                h0 = nc.dram_tensor(f"h0{k}", [npad[k], HID], f16)
                ag_out[k] = [nc.dram_tensor(f"ag{k}{l}", [npad[k], HID], f16,
                                            addr_space="Shared")
                             for l in range(2)]
                h_full[k] = [h0, ag_out[k][0], ag_out[k][1]]
                hT_loc[k] = [nc.dram_tensor(f"hT{k}{l}", [HID, tpc[k] * P], f16)
                             for l in range(3)]
                shard_buf[k] = [nc.dram_tensor(f"sh{k}{l}", [tpc[k] * P, HID], f16)
                                for l in range(2)]

            from concourse.masks import make_identity
            ident = wpool.tile([P, P], f16)
            make_identity(nc, ident[:])
            iota128 = wpool.tile([P, P], f16)
            nc.sync.dma_start(iota128[:], iota128_t[:])
            iotarep = wpool.tile([P, _OH_BATCH * P], f16)
            nc.sync.dma_start(iotarep[:], iotarep_t[:])
            iotaT = {}
            for dk in _DST_ETYPES:
                iotaT[dk] = wpool.tile([P, _STRIPE_T[dk] * P], f16,
                                       tag=f"iotaT{dk}", name=f"iotaT{dk}")
                nc.sync.dma_start(iotaT[dk][:], iotaT_t[dk][:])
            wclsr2 = wpool.tile([P, TC * HID], f16)
            nc.sync.dma_start(wclsr2[:], wclsr_t[:])
            ones1 = wpool.tile([1, P], f16)
            nc.gpsimd.memset(ones1[:], 1.0)
            win_sb = {}
            for k in tpc:
                win_sb[k] = wpool.tile([din[k], HID], f16, tag=f"win{k}", name=f"win{k}")
                nc.sync.dma_start(win_sb[k][:], win_t[k][:])
            wl_sb, wrc_sb, bc_sb = {}, {}, {}
            for et in _ETYPES:
                for l in range(NLAYERS):
                    wl_sb[(et, l)] = wpool.tile([HID, HID], f16, tag=f"wl{et}{l}",
                                                name=f"wl{et}{l}")
                    nc.sync.dma_start(wl_sb[(et, l)][:], wl_t[et][l])
            for dk in _DST_ETYPES:
                for l in range(NLAYERS):
                    wrc_sb[(dk, l)] = wpool.tile([HID, HID], f16, tag=f"wrc{dk}{l}",
                                                 name=f"wrc{dk}{l}")
                    nc.sync.dma_start(wrc_sb[(dk, l)][:], wrc_t[dk][l])
                    if bias_nonzero[dk]:
                        bc_sb[(dk, l)] = wpool.tile([1, HID], f16, tag=f"bc{dk}{l}",
                                                    name=f"bc{dk}{l}")
                        nc.sync.dma_start(bc_sb[(dk, l)][:], bc_t[dk][l])
            ivc_sb = {}
            for et in _ETYPES:
                ivc_sb[et] = wpool.tile([P, tpc[_ETYPES[et][0]]], f32,
                                        tag=f"ivc{et}", name=f"ivc{et}")
                nc.sync.dma_start(ivc_sb[et][:], ivc_t[et][:])

            # ---- input projection (full, replicated) ----
            GB = 4

            def proj_full_gen(k):
                # emit in piece-major position order; h0 is stored in the
                # same piece-major layout the AllGather'd tables use
                S = tpc[k] * P
                runs = []
                base_t = 0
                for (r0, r1, _es) in pieces[k]:
                    sz_t = (r1 - r0) // P
                    for c in range(N_CORES):
                        runs.append((base_t + c * sz_t,
                                     (c * S + r0) // P, sz_t))
                    base_t += N_CORES * sz_t
                for pos_t0, nat_t0, nt_ in runs:
                    for g in range(0, nt_, GB):
                        gn = min(GB, nt_ - g)
                        xt = gpool.tile([din[k], GB * P], f16, tag="xt")
                        nc.sync.dma_start(
                            xt[:, :gn * P],
                            din_t[k][:, (nat_t0 + g) * P:(nat_t0 + g + gn) * P])
                        zp = psum.tile([P, GB * HID], f32, space="PSUM", tag="zmr")
                        for j in range(gn):
                            nc.tensor.matmul(
                                out=zp[:, j * HID:(j + 1) * HID],
                                lhsT=xt[:, j * P:(j + 1) * P],
                                rhs=win_sb[k][:], start=(j == 0),
                                stop=(j == gn - 1),
                                skip_group_check=True)
                        zr = pool.tile([P, GB * HID], f16, tag="zr")
                        nc.scalar.activation(
                            out=zr[:, :gn * HID], in_=zp[:, :gn * HID],
                            func=mybir.ActivationFunctionType.Relu)
                        oap = h_full[k][0][:][
                            (pos_t0 + g) * P:(pos_t0 + g + gn) * P, :].rearrange(
                            "(t p) f -> p t f", p=P)
                        nc.sync.dma_start(oap, zr[:, :gn * HID].rearrange(
                            "p (t f) -> p t f", f=HID))
                        yield

            def proj_full(k):
                for _ in proj_full_gen(k):
                    pass

            def proj_own(k, GB=4):
                for g0 in range(0, tpc[k], GB):
                    gn = min(GB, tpc[k] - g0)
                    xt = gpool.tile([din[k], 8 * P], f16, tag="xt")
                    nc.sync.dma_start(xt[:, :gn * P],
                                      xown_t[k][:, g0 * P:(g0 + gn) * P])
                    ztp = psum.tile([HID, GB * P], f32, space="PSUM",
                                    tag=f"agg{(g0 // GB) % 2}", bufs=1)
                    for j in range(gn):
                        nc.tensor.matmul(
                            out=ztp[:, j * P:(j + 1) * P],
                            lhsT=win_sb[k][:],
                            rhs=xt[:, j * P:(j + 1) * P], start=(j == 0),
                            stop=(j == gn - 1), skip_group_check=True)
                    ztr = pool.tile([HID, GB * P], f16, tag="ztr")
                    nc.scalar.activation(
                        out=ztr[:, :gn * P], in_=ztp[:, :gn * P],
                        func=mybir.ActivationFunctionType.Relu)
                    nc.sync.dma_start(hT_loc[k][0][:, g0 * P:(g0 + gn) * P],
                                      ztr[:, :gn * P])

            # Emit projections in the order the first c-dst stripes consume
            # them: course pair-bank 0 (first ~65536 rows), then jockey, then
            # the rest of course. The big cheval projection is interleaved
            # into the c-dst stripe loop below.
            gen_r = proj_full_gen("r")
            nb0 = min(_ceil(2 * PBANK, P * GB), _ceil(npad["r"] // P, GB))
            for _ in range(nb0):
                next(gen_r, None)
            proj_full("j")
            for _ in gen_r:
                pass
            proj_own("j")
            proj_own("r")
            proj_own("c")

            # max gather-tile widths (chunks) per dst: tagA = primary et,
            # tagB = second et of dst c
            gta_ch = max(prep[et]["max_ch"]
                         for et in ("rev_part", "part", "rev_monte"))
            gtb_ch = prep["monte"]["max_ch"]

            qrot = [0]

            def do_stripe(dk, l, si, last):
                ets = _DST_ETYPES[dk]
                stripes = prep[ets[0]]["stripes"]
                ts, nt = stripes[si]
                nsub = _ceil(nt, 4)
                aggs = {}
                for eti, et in enumerate(ets):
                    pr = prep[et]
                    q = pr["q"]
                    cpt = pr["cpt"]
                    rlist = [r for r in pr["runs"] if r["si"] == si]
                    tag = "A" if eti == 0 else "B"
                    mx_ch = gta_ch if eti == 0 else gtb_ch
                    # per (sub) matmul counting for start/stop flags
                    mm_total = [0] * nsub
                    mm_seen = [0] * nsub
                    for r in rlist:
                        for (p2, t_rel) in r["norm_map"]:
                            mm_total[t_rel // 4] += 1
                        for _ in range(r["novf"] // P):
                            for p2 in (0, 1):
                                for t_rel in r["ovf_tiles"][p2]:
                                    mm_total[t_rel // 4] += 1
                    agg = []
                    for s in range(nsub):
                        atag = (f"agg{eti * 2 + s}" if nsub <= 2
                                else f"agg{s}")
                        agg.append(psum.tile([HID, 4 * P], f32, space="PSUM",
                                             tag=atag, name=atag, bufs=1))
                    aggs[et] = agg
                    for r in rlist:
                        ch = r["ch"]
                        n_slots = r["n_slots"]
                        wofs, cofs = r["wofs"], r["cofs"]
                        it = gpool.tile([P, mx_ch * P // 16],
                                        mybir.dt.int16, tag=f"it{tag}")
                        nc.sync.dma_start(it[:, :n_slots // 16],
                                          idx_t[et][:, wofs:wofs + n_slots // 16])
                        dch = r["dch"]
                        dt_ = gpool.tile([P, pr["max_dch"]], f16, tag=f"dt{tag}")
                        nc.sync.dma_start(dt_[:, :dch],
                                          dstl_t[et][:, cofs:cofs + dch])
                        gt = gpool.tile([P, mx_ch * P], f16, tag=f"gt{tag}")
                        sk = _ETYPES[et][1]
                        tab = h_full[sk][l]
                        b = r["b"]
                        pair_lo = b * PBANK
                        pair_hi = min((b + 1) * PBANK, npad[sk] // 2)
                        in_ap = tab[:][2 * pair_lo:2 * pair_hi, :].rearrange(
                            "(pr two) f -> pr (two f)", two=2)
                        nc.gpsimd.dma_gather(
                            out_ap=gt[:, :ch * P].rearrange(
                                "p (c f) -> p c f", f=P),
                            in_ap=in_ap,
                            idxs_ap=it[:, :n_slots // 16],
                            num_idxs=n_slots, num_idxs_reg=n_slots,
                            elem_size=P, single_packet=False,
                            queue_num=qrot[0] % 4)
                        qrot[0] += 1
                        # --- batched one-hot + matmuls: normal chunks ---
                        nnorm = len(r["norm_map"])
                        for c0 in range(0, nnorm, _OH_BATCH):
                            cn = min(_OH_BATCH, nnorm - c0)
                            oh = ohpool.tile([P, _OH_BATCH * P], f16,
                                             tag=f"oh{tag}")
                            nc.vector.tensor_tensor(
                                out=oh[:, :cn * P].rearrange(
                                    "p (c d) -> p c d", d=P),
                                in0=dt_[:, c0:c0 + cn].rearrange(
                                    "p (c u) -> p c u", u=1).to_broadcast(
                                    [P, cn, P]),
                                in1=iotarep[:, :cn * P].rearrange(
                                    "p (c d) -> p c d", d=P),
                                op=mybir.AluOpType.is_equal)
                            for cc in range(cn):
                                ci = c0 + cc
                                p2, t_rel = r["norm_map"][ci]
                                s = t_rel // 4
                                mm_seen[s] += 1
                                nc.tensor.matmul(
                                    out=agg[s][:, (t_rel % 4) * P:
                                               (t_rel % 4 + 1) * P],
                                    lhsT=gt[:, ci * P + p2 * HID:
                                            ci * P + p2 * HID + HID],
                                    rhs=oh[:, cc * P:(cc + 1) * P],
                                    start=(mm_seen[s] == 1),
                                    stop=(mm_seen[s] == mm_total[s]),
                                    skip_group_check=True)
                        # --- shared overflow chunks (labels span nt tiles,
                        # one gather chunk, two per-parity dstl columns) ---
                        for k2 in range(r["novf"] // P):
                            ci = nnorm + k2
                            for p2 in (0, 1):
                                if not r["ovf_tiles"][p2]:
                                    continue
                                dcol = nnorm + 2 * k2 + p2
                                oh = ohpool.tile([P, _STRIPE_T[dk] * P], f16,
                                                 tag=f"ohovf{tag}")
                                nc.vector.tensor_tensor(
                                    out=oh[:, :nt * P],
                                    in0=dt_[:, dcol:dcol + 1].to_broadcast(
                                        [P, nt * P]),
                                    in1=iotaT[dk][:, :nt * P],
                                    op=mybir.AluOpType.is_equal)
                                for t_rel in r["ovf_tiles"][p2]:
                                    s = t_rel // 4
                                    mm_seen[s] += 1
                                    nc.tensor.matmul(
                                        out=agg[s][:, (t_rel % 4) * P:
                                                   (t_rel % 4 + 1) * P],
                                        lhsT=gt[:, ci * P + p2 * HID:
                                                ci * P + p2 * HID + HID],
                                        rhs=oh[:, t_rel * P:(t_rel + 1) * P],
                                        start=(mm_seen[s] == 1),
                                        stop=(mm_seen[s] == mm_total[s]),
                                        skip_group_check=True)
                    assert mm_seen == mm_total, (dk, l, si, et, mm_seen, mm_total)
                # --- epilogue: aggs -> z -> relu -> stores ---
                aggsb = {}
                for et in ets:
                    a = pool.tile([HID, _STRIPE_T[dk] * P], f16, tag=f"aggsb{et}",
                                  name=f"aggsb{et}")
                    for s in range(nsub):
                        w = min(4 * P, (nt - s * 4) * P)
                        nc.scalar.activation(
                            out=a[:, s * 4 * P:s * 4 * P + w],
                            in_=aggs[et][s][:, :w],
                            func=mybir.ActivationFunctionType.Copy)
                    aggsb[et] = a
                hTs = pool.tile([HID, _STRIPE_T[dk] * P], f16, tag="hTs")
                nc.sync.dma_start(hTs[:, :nt * P],
                                  hT_loc[dk][l][:][:, ts * P:(ts + nt) * P])
                zsb = pool.tile([P, _STRIPE_T[dk] * HID], f16, tag="zsb")
                for j in range(nt):
                    t = ts + j
                    zmr = psum.tile([P, 4 * HID], f32, space="PSUM", tag="zmr")
                    for ei_, et in enumerate(ets):
                        nc.tensor.matmul(
                            out=zmr[:, ei_ * HID:(ei_ + 1) * HID],
                            lhsT=aggsb[et][:, j * P:(j + 1) * P],
                            rhs=wl_sb[(et, l)][:],
                            start=(ei_ == 0), stop=False,
                            skip_group_check=True)
                    ro = 2 * HID
                    nc.tensor.matmul(out=zmr[:, ro:ro + HID],
                                     lhsT=hTs[:, j * P:(j + 1) * P],
                                     rhs=wrc_sb[(dk, l)][:],
                                     start=False,
                                     stop=not bias_nonzero[dk],
                                     skip_group_check=True)
                    if bias_nonzero[dk]:
                        nc.tensor.matmul(out=zmr[:, ro:ro + HID],
                                         lhsT=ones1[:],
                                         rhs=bc_sb[(dk, l)][:],
                                         start=False, stop=True,
                                         skip_group_check=True)
                    # z = sum_et ivc_et * zm_et + zroot, then relu (fp16 out)
                    zrt = pool.tile([P, HID], f32, tag="zrt")
                    nc.scalar.activation(
                        out=zrt[:], in_=zmr[:, ro:ro + HID],
                        func=mybir.ActivationFunctionType.Copy)
                    tmp = pool.tile([P, HID], f32, tag="ztmp")
                    nc.vector.scalar_tensor_tensor(
                        out=tmp[:],
                        in0=zmr[:, 0:HID],
                        scalar=ivc_sb[ets[0]][:, t:t + 1],
                        in1=zrt[:],
                        op0=mybir.AluOpType.mult,
                        op1=mybir.AluOpType.add)
                    if len(ets) > 1:
                        nc.vector.scalar_tensor_tensor(
                            out=tmp[:],
                            in0=zmr[:, HID:2 * HID],
                            scalar=ivc_sb[ets[1]][:, t:t + 1],
                            in1=tmp[:],
                            op0=mybir.AluOpType.mult,
                            op1=mybir.AluOpType.add)
                    nc.scalar.activation(
                        out=zsb[:, j * HID:(j + 1) * HID], in_=tmp[:],
                        func=mybir.ActivationFunctionType.Relu)
                    if not last:
                        # transposed copy for next layer's root term
                        ztp = psum.tile([HID, P], f16, space="PSUM", tag="ztp")
                        nc.tensor.transpose(
                            out=ztp[:, :],
                            in_=zsb[:, j * HID:(j + 1) * HID],
                            identity=ident[:])
                        ztr2 = pool.tile([HID, P], f16, tag="ztr2")
                        nc.scalar.activation(
                            out=ztr2[:], in_=ztp[:, :],
                            func=mybir.ActivationFunctionType.Copy)
                        nc.sync.dma_start(
                            hT_loc[dk][l + 1][:][:, t * P:(t + 1) * P],
                            ztr2[:])
                if not last:
                    if not (dk == "c" and l == NLAYERS - 2):
                        oap = shard_buf[dk][l][:][ts * P:(ts + nt) * P, :].rearrange(
                            "(t p) f -> p t f", p=P)
                        nc.sync.dma_start(oap, zsb[:, :nt * HID].rearrange(
                            "p (t f) -> p t f", f=HID))
                else:
                    tmp2 = pool.tile([P, TC * HID], f16, tag="ctmp")
                    nc.vector.tensor_tensor(
                        out=tmp2[:, :nt * HID], in0=zsb[:, :nt * HID],
                        in1=wclsr2[:, :nt * HID],
                        op=mybir.AluOpType.mult)
                    ot = pool.tile([P, TC], f32, tag="otile")
                    nc.vector.tensor_reduce(
                        out=ot[:, :nt],
                        in_=tmp2[:, :nt * HID].rearrange(
                            "p (t f) -> p t f", f=HID),
                        axis=mybir.AxisListType.X,
                        op=mybir.AluOpType.add)
                    if b_cls != 0.0:
                        nc.vector.tensor_scalar(
                            out=ot[:, :nt], in0=ot[:, :nt],
                            scalar1=b_cls, scalar2=None,
                            op0=mybir.AluOpType.add)
                    oap = out_t[:].rearrange("(t p) o -> p t o", p=P)
                    nc.sync.dma_start(oap[:, ts:ts + nt, 0], ot[:, :nt])

            # AllGathers are chunked into ~10MB pieces. Pieces whose producer
            # stripes finished ≥2 stripes ago are issued IN-phase (their row
            # slice is already stored, so the in-order Pool queue does not
            # stall); the last piece is deferred into the next dst phase so
            # the collective never head-blocks the next phase's first gathers
            # behind this phase's epilogue tail.
            pending_ag = []  # [(table_key, closure)]

            def ag_piece(dk, l, r0, r1, gbase):
                def _ag():
                    nc.gpsimd.collective_compute(
                        "AllGather", mybir.AluOpType.bypass,
                        ins=[shard_buf[dk][l][:][r0:r1, :]],
                        outs=[ag_out[dk][l][:][
                            gbase:gbase + N_CORES * (r1 - r0), :]],
                        replica_groups=[list(range(N_CORES))])
                return _ag

            def require_tables(dk, l):
                # force-issue any still-pending AllGather pieces whose output
                # tables this phase's gathers read (safety: Tile does NOT
                # reorder a later collective before an earlier read)
                if l == 0:
                    return
                need = {(_ETYPES[et][1], l - 1) for et in _DST_ETYPES[dk]}
                i = 0
                while i < len(pending_ag):
                    key, fn = pending_ag[i]
                    if key in need:
                        fn()
                        pending_ag.pop(i)
                    else:
                        i += 1

            def do_dst(dk, l, interleave=None, per_stripe=0):
                last = (l == NLAYERS - 1)
                require_tables(dk, l)
                stripes = prep[_DST_ETYPES[dk][0]]["stripes"]
                nstr = len(stripes)
                own_pieces = {}
                deferred = []
                if not last and not (dk == "c" and l == NLAYERS - 2):
                    gbase = 0
                    for (r0, r1, es) in pieces[dk]:
                        piece = ag_piece(dk, l, r0, r1, gbase)
                        gbase += N_CORES * (r1 - r0)
                        trig = es + 1  # 2-stripe lag behind producer
                        if trig <= nstr - 2:
                            own_pieces.setdefault(trig, []).append(piece)
                        else:
                            deferred.append(((dk, l), piece))
                for si in range(nstr):
                    do_stripe(dk, l, si, last)
                    if pending_ag and si >= 1:
                        pending_ag.pop(0)[1]()
                    for fn in own_pieces.pop(si, []):
                        fn()
                    if interleave is not None:
                        for _ in range(per_stripe):
                            if next(interleave, "done") == "done":
                                interleave = None
                                break
                if interleave is not None:
                    for _ in interleave:
                        pass
                pending_ag.extend(deferred)

            order = {0: ["c", "r", "j"], 1: ["r", "j", "c"], 2: ["c"]}
            for l in range(NLAYERS):
                for dk in order[min(l, 2)] if NLAYERS == 3 else ["c", "r", "j"]:
                    if l == NLAYERS - 1 and dk != "c":
                        continue
                    if l == 0 and dk == "c":
                        # cheval projection interleaved among the c-dst l0
                        # stripes so h0_c is ready when r-dst l0 starts
                        nstr = len(prep[_DST_ETYPES["c"][0]]["stripes"])
                        ngroups = _ceil(npad["c"] // P, GB)
                        do_dst(dk, l, interleave=proj_full_gen("c"),
                               per_stripe=_ceil(ngroups, nstr))
                    else:
                        do_dst(dk, l)
            for _, fn in pending_ag:
                fn()
            pending_ag.clear()

    nc.finalize()

    iota128_v = np.broadcast_to(np.arange(P, dtype=np.float16), (P, P)).copy()
    iotarep_v = np.broadcast_to(
        np.tile(np.arange(P, dtype=np.float16), _OH_BATCH),
        (P, _OH_BATCH * P)).copy()
    iotaT_v = {dk: np.broadcast_to(
        np.arange(_STRIPE_T[dk] * P, dtype=np.float16),
        (P, _STRIPE_T[dk] * P)).copy() for dk in _DST_ETYPES}
    wclsr_v = np.tile(w_cls.reshape(1, HID), (P, TC)).astype(np.float16)

    in_maps = []
    for c in range(N_CORES):
        m = {}
        for k in tpc:
            sh = tpc[k] * P
            m[f"xT_{k}"] = xT[k]
            m[f"xo_{k}"] = np.ascontiguousarray(xT[k][:, c * sh:(c + 1) * sh])
            m[f"win_{k}"] = w_in_np[k]
        for et in _ETYPES:
            m[f"idx_{et}"] = prep[et]["idx"][c]
            m[f"dstl_{et}"] = prep[et]["dstl"][c]
            m[f"ivc_{et}"] = np.ascontiguousarray(iv[et][c].T)
            m[f"wl_{et}"] = WL[et]
        for dk in _DST_ETYPES:
            m[f"wrc_{dk}"] = WRc[dk]
            m[f"bc_{dk}"] = Bc[dk].reshape(NLAYERS, 1, HID)
            m[f"iotaT_{dk}"] = iotaT_v[dk]
        m["iota128"] = iota128_v
        m["iotarep"] = iotarep_v
        m["wclsr"] = wclsr_v
        in_maps.append(m)

    install_neuronx_cc_hook()
    partition_name = nc.partition_id_tensor.name if nc.partition_id_tensor else None
    in_names, out_names, out_avals, zero_outs = [], [], [], []
    for alloc in nc.m.functions[0].allocations:
        if not isinstance(alloc, mybir.MemoryLocationSet):
            continue
        name = alloc.memorylocations[0].name
        if alloc.kind == "ExternalInput":
            if name != partition_name:
                in_names.append(name)
        elif alloc.kind == "ExternalOutput":
            out_names.append(name)
            shape = tuple(alloc.tensor_shape)
            dtype = mybir.dt.np(alloc.dtype)
            out_avals.append(jax.core.ShapedArray(shape, dtype))
            zero_outs.append(np.zeros(shape, dtype))
    n_params = len(in_names)
    all_in = list(in_names) + list(out_names)
    if partition_name is not None:
        all_in.append(partition_name)

    def _body(*args):
        operands = list(args)
        if partition_name is not None:
            operands.append(partition_id_tensor())
        outs = _bass_exec_p.bind(
            *operands, out_avals=tuple(out_avals), in_names=tuple(all_in),
            out_names=tuple(out_names), lowering_input_output_aliases=(),
            sim_require_finite=False, sim_require_nnan=False, nc=nc)
        return tuple(outs)

    devices = jax.devices()[:N_CORES]
    mesh = Mesh(np.asarray(devices), ("core",))
    specs = (PartitionSpec("core"),)
    sharded = jax.jit(
        shard_map(_body, mesh=mesh, in_specs=specs * (n_params + len(out_names)),
                  out_specs=specs * len(out_names), check_rep=False),
        keep_unused=True)
    per_core = [[np.asarray(m[n]) for n in in_names] for m in in_maps]
    concat_in = [np.concatenate([per_core[c][i] for c in range(N_CORES)], axis=0)
                 for i in range(n_params)]
    concat_zero = [np.zeros((N_CORES * z.shape[0], *z.shape[1:]), z.dtype)
                   for z in zero_outs]
    shd = NamedSharding(mesh, PartitionSpec("core"))
    dev_in = [
        jax.make_array_from_callback(a.shape, shd, lambda idx, a=a: a[idx])
        for a in concat_in + concat_zero
    ]
    outs = sharded(*dev_in)
    jax.block_until_ready(outs)
    global _PROF
    _PROF = dict(sharded=sharded, dev_in=dev_in, nc=nc, out_names=out_names)
    oi = out_names.index("out")
    full = np.asarray(outs[oi]).reshape(N_CORES * tpc["c"] * P, 1)
    return full[:NC, :].astype(np.float32)



# revision 21
# speedup vs baseline: 1.4024x; 1.2776x over previous
"""Trainium2 Bass kernel for 3-layer heterogeneous GraphSAGE (EntityGraphNN).

8 NeuronCores, SPMD single program, fp16 data path (f32 PSUM accumulate):
  - Destination-node sharding: each core owns 1/8 of each node type's
    128-row tiles. Edges routed to the core owning their dst.
  - h tables stored fp16 [N, 64]; gathers fetch PAIRS of rows (256B) via
    gpsimd dma_gather. Pair-index space is banked by 32768 (int16 limit);
    cells are (tile, pairbank, src-parity) so every 128-slot chunk is
    parity-pure and the matmul lhsT picks the right 64-wide half.
  - Scatter-add via one-hot matmul: oh[e,d] = (dstl[e]==d) built on DVE in
    BATCHED 3-D broadcast ops (one instr per <=16 chunks), fp16; PSUM
    agg_T[64, 512] accumulates G^T @ oh per 4-tile sub-stripe.
  - z = (agg@Wl)*invcnt + h_dst@Wr (+b); ReLU; store h fp16 row-major
    (shard -> AllGather table) and transposed (next layer's root term).
  - Input projections replicated on every core (no layer-0 AllGather).
  - AllGather per dst type right after its stripes finish; layer order
    c,r,j then r,j,c then c, so each AG overlaps the following compute.
    AG of cheval after layer 1 is never consumed -> skipped.
  - No global barriers: Tile's shadow-memory dep tracking orders the
    collectives against producers/consumers.
"""
import numpy as np

HID = 64
P = 128
PBANK = 32768  # pairs per gather bank (int16 index range)
N_CORES = 8
OVF_CAP = 1024  # max shared overflow slots per run before q escalation

_ETYPES = {
    "rev_part": ("c", "r"),
    "monte": ("c", "j"),
    "part": ("r", "c"),
    "rev_monte": ("j", "c"),
}
_DST_ETYPES = {"c": ["rev_part", "monte"], "r": ["part"], "j": ["rev_monte"]}
_STRIPE_T = {"c": 8, "r": 16, "j": 8}
_OH_BATCH = 16  # chunks per batched one-hot DVE op


def _ceil(a, b):
    return (a + b - 1) // b


def _prep_edge_type(ei, n_src_pad, tpc, q, T):
    """Pack edges into (stripe, pairbank) gather runs with parity-pure
    128-slot chunks. Returns None if any (run, parity) overflow exceeds
    OVF_CAP (caller escalates q)."""
    src = ei[0].astype(np.int64)
    dst = ei[1].astype(np.int64)
    npair = n_src_pad // 2
    nbp = _ceil(npair, PBANK)
    shard = tpc * P
    core = dst // shard
    tile = (dst % shard) // P
    pair = src >> 1
    par = src & 1
    pb = pair // PBANK

    stripes = []
    t0 = 0
    while t0 < tpc:
        stripes.append((t0, min(T, tpc - t0)))
        t0 += T

    order = np.lexsort((src, par, pb, tile, core))
    so, do_ = src[order], dst[order]
    key = ((core * tpc + tile) * nbp + pb) * 2 + par
    ko = key[order]
    ncell = N_CORES * tpc * nbp * 2
    cnts = np.bincount(key, minlength=ncell).reshape(N_CORES, tpc, nbp, 2)
    starts = np.zeros(ncell + 1, np.int64)
    np.cumsum(cnts.reshape(-1), out=starts[1:])

    cpt = q // P
    runs = []
    total_slots = 0
    total_dcols = 0
    for si, (ts, nt) in enumerate(stripes):
        for b in range(nbp):
            # shared (both-parity) overflow block, 128-padded
            oc_max = 0
            ovf_tiles = [set(), set()]
            for c in range(N_CORES):
                oc = 0
                for p2 in (0, 1):
                    ov = np.maximum(cnts[c, ts:ts + nt, b, p2] - q, 0)
                    oc += int(ov.sum())
                    for t_rel in np.nonzero(ov)[0]:
                        ovf_tiles[p2].add(int(t_rel))
                oc_max = max(oc_max, oc)
            if oc_max > OVF_CAP:
                return None
            novf = _ceil(oc_max, P) * P
            n_slots = 2 * nt * q + novf
            norm_map = [(p2, t_rel)
                        for p2 in (0, 1)
                        for t_rel in range(nt)
                        for _ in range(cpt)]
            runs.append(dict(si=si, b=b, ts=ts, nt=nt, n_slots=n_slots,
                             novf=novf,
                             ovf_tiles=(sorted(ovf_tiles[0]),
                                        sorted(ovf_tiles[1])),
                             norm_map=norm_map))
            total_slots += n_slots
            # dstl columns: 1 per normal chunk, 2 per shared overflow chunk
            total_dcols += len(norm_map) + 2 * (novf // P)

    w_tot = total_slots // 16
    idx_all = np.zeros((N_CORES, 16, w_tot), np.int16)
    dstl_all = np.full((N_CORES, P, total_dcols), -1.0, np.float16)

    wofs = 0
    cofs = 0
    for r in runs:
        ts, nt, b = r["ts"], r["nt"], r["b"]
        n_slots = r["n_slots"]
        novf = r["novf"]
        nnorm = len(r["norm_map"])
        for c in range(N_CORES):
            idxs = np.zeros(n_slots, np.int64)
            # normal-chunk labels + per-parity overflow labels
            dls = np.full((n_slots // P, P), -1.0, np.float32)
            ovf_lab = [np.full(novf, -1.0, np.float32) for _ in (0, 1)]
            base_ovf = 2 * nt * q
            ovf_off = 0
            for p2 in (0, 1):
                for t_rel in range(nt):
                    t = ts + t_rel
                    k = ((c * tpc + t) * nbp + b) * 2 + p2
                    s0, s1 = int(starts[k]), int(starts[k + 1])
                    cell_pair = (so[s0:s1] >> 1) - b * PBANK
                    cell_dst = do_[s0:s1] - (c * shard + t * P)
                    take = min(s1 - s0, q)
                    blk = (p2 * nt + t_rel) * q
                    idxs[blk:blk + take] = cell_pair[:take]
                    dls.reshape(-1)[blk:blk + take] = cell_dst[:take]
                    if s1 - s0 > take:
                        no = s1 - s0 - take
                        assert ovf_off + no <= novf
                        idxs[base_ovf + ovf_off:base_ovf + ovf_off + no] = \
                            cell_pair[take:]
                        ovf_lab[p2][ovf_off:ovf_off + no] = \
                            cell_dst[take:] + t_rel * P
                        ovf_off += no
            idx_all[c, :, wofs:wofs + n_slots // 16] = \
                idxs.reshape(n_slots // 16, 16).T
            # dstl layout: [normal chunks][ovf chunk p0, ovf chunk p1]...
            dcols = np.full((total_dcols_run(r), P), -1.0, np.float32)
            dcols[:nnorm] = dls[:nnorm]
            for k2 in range(novf // P):
                dcols[nnorm + 2 * k2] = ovf_lab[0][k2 * P:(k2 + 1) * P]
                dcols[nnorm + 2 * k2 + 1] = ovf_lab[1][k2 * P:(k2 + 1) * P]
            dstl_all[c, :, cofs:cofs + dcols.shape[0]] = \
                dcols.T.astype(np.float16)
        r["wofs"] = wofs
        r["cofs"] = cofs
        r["ch"] = n_slots // P
        r["dch"] = total_dcols_run(r)
        wofs += n_slots // 16
        cofs += r["dch"]

    idx_rep = np.tile(idx_all, (1, 8, 1))
    return dict(runs=runs, stripes=stripes, idx=idx_rep, dstl=dstl_all,
                nbp=nbp, q=q, cpt=cpt, n_chunks=total_dcols, w_tot=w_tot,
                max_ch=max(r["ch"] for r in runs),
                max_dch=max(r["dch"] for r in runs))


def total_dcols_run(r):
    return len(r["norm_map"]) + 2 * (r["novf"] // P)


def _prep_type(ei, n_src_pad, tpc, T):
    nbp = _ceil(n_src_pad // 2, PBANK)
    lam_cell = ei.shape[1] / (N_CORES * tpc) / (nbp * 2)
    q = max(P, (int(lam_cell) // P) * P)
    while True:
        r = _prep_edge_type(ei, n_src_pad, tpc, q, T)
        if r is not None:
            return r
        q += P


def _invcnt(ei, n_dst_pad, tpc):
    cnt = np.bincount(ei[1].astype(np.int64), minlength=n_dst_pad).astype(np.float32)
    inv = 1.0 / np.maximum(cnt, 1.0)
    return inv.reshape(N_CORES, tpc, P)


def kernel(**inputs):
    import concourse.bass as bass
    import concourse.mybir as mybir
    import concourse.tile as tile
    import concourse.bacc as bacc
    import jax
    from jax.sharding import Mesh, PartitionSpec, NamedSharding
    from jax.experimental.shard_map import shard_map
    from concourse.bass2jax import (_bass_exec_p, partition_id_tensor,
                                    install_neuronx_cc_hook)

    f32 = mybir.dt.float32
    f16 = mybir.dt.float16
    x_np = {"c": np.asarray(inputs["x_cheval"], np.float32),
            "j": np.asarray(inputs["x_jockey"], np.float32),
            "r": np.asarray(inputs["x_course"], np.float32)}
    NC = x_np["c"].shape[0]

    tpc = {k: _ceil(x_np[k].shape[0], P * N_CORES) for k in x_np}
    npad = {k: tpc[k] * P * N_CORES for k in tpc}
    din = {k: x_np[k].shape[1] + 1 for k in x_np}  # +1 ones row folds bias

    xT = {}
    for k in x_np:
        xt = np.zeros((din[k], npad[k]), np.float16)
        xt[:-1, :x_np[k].shape[0]] = x_np[k].T.astype(np.float16)
        xt[-1, :] = 1.0
        xT[k] = xt

    w_in_np = {}
    for k, nm in (("c", "cheval"), ("j", "jockey"), ("r", "course")):
        w = np.asarray(inputs[f"w_in_{nm}"], np.float32)
        b = np.asarray(inputs[f"b_in_{nm}"], np.float32)
        w_in_np[k] = np.concatenate([w, b.reshape(1, HID)], axis=0).astype(np.float16)

    w_cls = np.asarray(inputs["w_cls"], np.float32)
    b_cls = float(np.asarray(inputs["b_cls"]).reshape(-1)[0])
    eis = {k: np.asarray(inputs["ei_" + k]) for k in _ETYPES}
    NLAYERS = np.asarray(inputs["wl_part"]).shape[0]

    # ---- piece-major table layout -------------------------------------
    # AllGathers are chunked; the BIR verifier requires contiguous
    # collective outputs, so tables are stored piece-major: for each row
    # piece [r0:r1) (aligned to stripe boundaries of the producing dst
    # phase), all 8 cores' rows are contiguous. Gather indices are
    # remapped host-side; dst-side structures stay in natural order.
    _N_PIECES = {"c": 4, "r": 2, "j": 1}

    def _mk_stripes(tpc_k, T):
        out = []
        t0 = 0
        while t0 < tpc_k:
            out.append((t0, min(T, tpc_k - t0)))
            t0 += T
        return out

    pieces = {}
    piecemap = {}
    for k in tpc:
        stripes_k = _mk_stripes(tpc[k], _STRIPE_T[k])
        nstr = len(stripes_k)
        npc = min(_N_PIECES[k], nstr)
        bounds = []
        r0 = 0
        for j in range(npc):
            es = ((j + 1) * nstr) // npc
            r1 = (stripes_k[es - 1][0] + stripes_k[es - 1][1]) * P
            if r1 > r0:
                bounds.append((r0, r1, es))
            r0 = r1
        pieces[k] = bounds
        S = tpc[k] * P
        pm = np.empty(npad[k], np.int64)
        base = 0
        for (r0, r1, _es) in bounds:
            sz = r1 - r0
            for c in range(N_CORES):
                pm[c * S + r0:c * S + r1] = base + c * sz + np.arange(sz)
            base += N_CORES * sz
        piecemap[k] = pm

    prep, iv = {}, {}
    for et, (dk, sk) in _ETYPES.items():
        ei_remap = np.stack([piecemap[sk][eis[et][0].astype(np.int64)],
                             eis[et][1].astype(np.int64)])
        prep[et] = _prep_type(ei_remap, npad[sk], tpc[dk], _STRIPE_T[dk])
        iv[et] = _invcnt(eis[et], npad[dk], tpc[dk])

    WL = {et: np.asarray(inputs["wl_" + et], np.float16) for et in _ETYPES}
    BL = {et: np.asarray(inputs["bl_" + et], np.float32) for et in _ETYPES}
    WR = {et: np.asarray(inputs["wr_" + et], np.float32) for et in _ETYPES}
    WRc = {dk: sum(WR[et] for et in _DST_ETYPES[dk]).astype(np.float16)
           for dk in _DST_ETYPES}
    Bc = {dk: sum(BL[et] for et in _DST_ETYPES[dk]).astype(np.float16)
          for dk in _DST_ETYPES}
    bias_nonzero = {dk: bool(np.any(Bc[dk])) for dk in _DST_ETYPES}

    nc = bacc.Bacc(None, num_swdge_queues=4, dynamic_dma_scratch_size=49152)

    din_t = {k: nc.declare_dram_parameter(f"xT_{k}", [din[k], npad[k]], f16, False)
             for k in tpc}
    xown_t = {k: nc.declare_dram_parameter(f"xo_{k}", [din[k], tpc[k] * P], f16, False)
              for k in tpc}
    win_t = {k: nc.declare_dram_parameter(f"win_{k}", [din[k], HID], f16, False)
             for k in tpc}
    idx_t = {et: nc.declare_dram_parameter(f"idx_{et}", [P, prep[et]["w_tot"]],
                                           mybir.dt.int16, False) for et in _ETYPES}
    dstl_t = {et: nc.declare_dram_parameter(f"dstl_{et}", [P, prep[et]["n_chunks"]],
                                            f16, False) for et in _ETYPES}
    ivc_t = {et: nc.declare_dram_parameter(f"ivc_{et}", [P, tpc[_ETYPES[et][0]]],
                                           f32, False) for et in _ETYPES}
    wl_t = {et: nc.declare_dram_parameter(f"wl_{et}", [NLAYERS, HID, HID], f16, False)
            for et in _ETYPES}
    wrc_t = {dk: nc.declare_dram_parameter(f"wrc_{dk}", [NLAYERS, HID, HID], f16,
                                           False) for dk in _DST_ETYPES}
    bc_t = {dk: nc.declare_dram_parameter(f"bc_{dk}", [NLAYERS, 1, HID], f16, False)
            for dk in _DST_ETYPES}
    iota128_t = nc.declare_dram_parameter("iota128", [P, P], f16, False)
    iotarep_t = nc.declare_dram_parameter("iotarep", [P, _OH_BATCH * P], f16, False)
    iotaT_t = {dk: nc.declare_dram_parameter(f"iotaT_{dk}", [P, _STRIPE_T[dk] * P],
                                             f16, False) for dk in _DST_ETYPES}
    wclsr_t = nc.declare_dram_parameter("wclsr", [P, _STRIPE_T["c"] * HID], f16, False)
    out_t = nc.declare_dram_parameter("out", [tpc["c"] * P, 1], f32, True)

    TC = _STRIPE_T["c"]

    with tile.TileContext(nc) as tc:
        with (
            tc.tile_pool(name="wpool", bufs=1) as wpool,
            tc.tile_pool(name="gpool", bufs=3) as gpool,
            tc.tile_pool(name="ohpool", bufs=2) as ohpool,
            tc.tile_pool(name="pool", bufs=2) as pool,
            tc.tile_pool(name="psum", bufs=2, space="PSUM") as psum,
        ):
            h_full = {}
            hT_loc = {}
            shard_buf = {}
            ag_out = {}
            for k in tpc:
                h0 = nc.dram_tensor(f"h0{k}", [npad[k], HID], f16)
                ag_out[k] = [nc.dram_tensor(f"ag{k}{l}", [npad[k], HID], f16,
                                            addr_space="Shared")
                             for l in range(2)]
                h_full[k] = [h0, ag_out[k][0], ag_out[k][1]]
                hT_loc[k] = [nc.dram_tensor(f"hT{k}{l}", [HID, tpc[k] * P], f16)
                             for l in range(3)]
                shard_buf[k] = [nc.dram_tensor(f"sh{k}{l}", [tpc[k] * P, HID], f16)
                                for l in range(2)]

            from concourse.masks import make_identity
            ident = wpool.tile([P, P], f16)
            make_identity(nc, ident[:])
            iota128 = wpool.tile([P, P], f16)
            nc.sync.dma_start(iota128[:], iota128_t[:])
            iotarep = wpool.tile([P, _OH_BATCH * P], f16)
            nc.sync.dma_start(iotarep[:], iotarep_t[:])
            iotaT = {}
            for dk in _DST_ETYPES:
                iotaT[dk] = wpool.tile([P, _STRIPE_T[dk] * P], f16,
                                       tag=f"iotaT{dk}", name=f"iotaT{dk}")
                nc.sync.dma_start(iotaT[dk][:], iotaT_t[dk][:])
            wclsr2 = wpool.tile([P, TC * HID], f16)
            nc.sync.dma_start(wclsr2[:], wclsr_t[:])
            ones1 = wpool.tile([1, P], f16)
            nc.gpsimd.memset(ones1[:], 1.0)
            win_sb = {}
            for k in tpc:
                win_sb[k] = wpool.tile([din[k], HID], f16, tag=f"win{k}", name=f"win{k}")
                nc.sync.dma_start(win_sb[k][:], win_t[k][:])
            wl_sb, wrc_sb, bc_sb = {}, {}, {}
            for et in _ETYPES:
                for l in range(NLAYERS):
                    wl_sb[(et, l)] = wpool.tile([HID, HID], f16, tag=f"wl{et}{l}",
                                                name=f"wl{et}{l}")
                    nc.sync.dma_start(wl_sb[(et, l)][:], wl_t[et][l])
            for dk in _DST_ETYPES:
                for l in range(NLAYERS):
                    wrc_sb[(dk, l)] = wpool.tile([HID, HID], f16, tag=f"wrc{dk}{l}",
                                                 name=f"wrc{dk}{l}")
                    nc.sync.dma_start(wrc_sb[(dk, l)][:], wrc_t[dk][l])
                    if bias_nonzero[dk]:
                        bc_sb[(dk, l)] = wpool.tile([1, HID], f16, tag=f"bc{dk}{l}",
                                                    name=f"bc{dk}{l}")
                        nc.sync.dma_start(bc_sb[(dk, l)][:], bc_t[dk][l])
            ivc_sb = {}
            for et in _ETYPES:
                ivc_sb[et] = wpool.tile([P, tpc[_ETYPES[et][0]]], f32,
                                        tag=f"ivc{et}", name=f"ivc{et}")
                nc.sync.dma_start(ivc_sb[et][:], ivc_t[et][:])

            # ---- input projection (full, replicated) ----
            GB = 4

            def proj_full_gen(k):
                # emit in piece-major position order; h0 is stored in the
                # same piece-major layout the AllGather'd tables use
                S = tpc[k] * P
                runs = []
                base_t = 0
                for (r0, r1, _es) in pieces[k]:
                    sz_t = (r1 - r0) // P
                    for c in range(N_CORES):
                        runs.append((base_t + c * sz_t,
                                     (c * S + r0) // P, sz_t))
                    base_t += N_CORES * sz_t
                for pos_t0, nat_t0, nt_ in runs:
                    for g in range(0, nt_, GB):
                        gn = min(GB, nt_ - g)
                        xt = gpool.tile([din[k], GB * P], f16, tag="xt")
                        nc.sync.dma_start(
                            xt[:, :gn * P],
                            din_t[k][:, (nat_t0 + g) * P:(nat_t0 + g + gn) * P])
                        zp = psum.tile([P, GB * HID], f32, space="PSUM", tag="zmr")
                        for j in range(gn):
                            nc.tensor.matmul(
                                out=zp[:, j * HID:(j + 1) * HID],
                                lhsT=xt[:, j * P:(j + 1) * P],
                                rhs=win_sb[k][:], start=(j == 0),
                                stop=(j == gn - 1),
                                skip_group_check=True)
                        zr = pool.tile([P, GB * HID], f16, tag="zr")
                        nc.scalar.activation(
                            out=zr[:, :gn * HID], in_=zp[:, :gn * HID],
                            func=mybir.ActivationFunctionType.Relu)
                        oap = h_full[k][0][:][
                            (pos_t0 + g) * P:(pos_t0 + g + gn) * P, :].rearrange(
                            "(t p) f -> p t f", p=P)
                        nc.sync.dma_start(oap, zr[:, :gn * HID].rearrange(
                            "p (t f) -> p t f", f=HID))
                        yield

            def proj_full(k):
                for _ in proj_full_gen(k):
                    pass

            def proj_own(k, GB=4):
                for g0 in range(0, tpc[k], GB):
                    gn = min(GB, tpc[k] - g0)
                    xt = gpool.tile([din[k], 8 * P], f16, tag="xt")
                    nc.sync.dma_start(xt[:, :gn * P],
                                      xown_t[k][:, g0 * P:(g0 + gn) * P])
                    ztp = psum.tile([HID, GB * P], f32, space="PSUM",
                                    tag=f"agg{(g0 // GB) % 2}", bufs=1)
                    for j in range(gn):
                        nc.tensor.matmul(
                            out=ztp[:, j * P:(j + 1) * P],
                            lhsT=win_sb[k][:],
                            rhs=xt[:, j * P:(j + 1) * P], start=(j == 0),
                            stop=(j == gn - 1), skip_group_check=True)
                    ztr = pool.tile([HID, GB * P], f16, tag="ztr")
                    nc.scalar.activation(
                        out=ztr[:, :gn * P], in_=ztp[:, :gn * P],
                        func=mybir.ActivationFunctionType.Relu)
                    nc.sync.dma_start(hT_loc[k][0][:, g0 * P:(g0 + gn) * P],
                                      ztr[:, :gn * P])

            # Emit projections in the order the first c-dst stripes consume
            # them: course pair-bank 0 (first ~65536 rows), then jockey, then
            # the rest of course. The big cheval projection is interleaved
            # into the c-dst stripe loop below.
            gen_r = proj_full_gen("r")
            nb0 = min(_ceil(2 * PBANK, P * GB), _ceil(npad["r"] // P, GB))
            for _ in range(nb0):
                next(gen_r, None)
            proj_full("j")
            for _ in gen_r:
                pass
            proj_own("j")
            proj_own("r")
            proj_own("c")

            # max gather-tile widths (chunks) per dst: tagA = primary et,
            # tagB = second et of dst c
            gta_ch = max(prep[et]["max_ch"]
                         for et in ("rev_part", "part", "rev_monte"))
            gtb_ch = prep["monte"]["max_ch"]

            qrot = [0]

            def do_stripe(dk, l, si, last):
                ets = _DST_ETYPES[dk]
                stripes = prep[ets[0]]["stripes"]
                ts, nt = stripes[si]
                nsub = _ceil(nt, 4)
                aggs = {}
                for eti, et in enumerate(ets):
                    pr = prep[et]
                    q = pr["q"]
                    cpt = pr["cpt"]
                    rlist = [r for r in pr["runs"] if r["si"] == si]
                    tag = "A" if eti == 0 else "B"
                    mx_ch = gta_ch if eti == 0 else gtb_ch
                    # per (sub) matmul counting for start/stop flags
                    mm_total = [0] * nsub
                    mm_seen = [0] * nsub
                    for r in rlist:
                        for (p2, t_rel) in r["norm_map"]:
                            mm_total[t_rel // 4] += 1
                        for _ in range(r["novf"] // P):
                            for p2 in (0, 1):
                                for t_rel in r["ovf_tiles"][p2]:
                                    mm_total[t_rel // 4] += 1
                    agg = []
                    for s in range(nsub):
                        atag = (f"agg{eti * 2 + s}" if nsub <= 2
                                else f"agg{s}")
                        agg.append(psum.tile([HID, 4 * P], f32, space="PSUM",
                                             tag=atag, name=atag, bufs=1))
                    aggs[et] = agg
                    for r in rlist:
                        ch = r["ch"]
                        n_slots = r["n_slots"]
                        wofs, cofs = r["wofs"], r["cofs"]
                        it = gpool.tile([P, mx_ch * P // 16],
                                        mybir.dt.int16, tag=f"it{tag}")
                        nc.sync.dma_start(it[:, :n_slots // 16],
                                          idx_t[et][:, wofs:wofs + n_slots // 16])
                        dch = r["dch"]
                        dt_ = gpool.tile([P, pr["max_dch"]], f16, tag=f"dt{tag}")
                        nc.sync.dma_start(dt_[:, :dch],
                                          dstl_t[et][:, cofs:cofs + dch])
                        gt = gpool.tile([P, mx_ch * P], f16, tag=f"gt{tag}")
                        sk = _ETYPES[et][1]
                        tab = h_full[sk][l]
                        b = r["b"]
                        pair_lo = b * PBANK
                        pair_hi = min((b + 1) * PBANK, npad[sk] // 2)
                        in_ap = tab[:][2 * pair_lo:2 * pair_hi, :].rearrange(
                            "(pr two) f -> pr (two f)", two=2)
                        # split into <=1024-slot sub-gathers so each engine's
                        # concatenated (single_packet) stream stays within the
                        # 64-descriptor packet ceiling; concatenation lets the
                        # SDMA m2s pipeline its 256B random reads instead of
                        # serializing one HBM latency per descriptor
                        SUB = 1024
                        for s0 in range(0, n_slots, SUB):
                            s1 = min(s0 + SUB, n_slots)
                            nc.gpsimd.dma_gather(
                                out_ap=gt[:, s0:_ceil(s1, P) * P].rearrange(
                                    "p (c f) -> p c f", f=P),
                                in_ap=in_ap,
                                idxs_ap=it[:, s0 // 16:_ceil(s1, 16)],
                                num_idxs=s1 - s0, num_idxs_reg=s1 - s0,
                                elem_size=P, single_packet=True,
                                queue_num=qrot[0] % 4)
                            qrot[0] += 1
                        # --- batched one-hot + matmuls: normal chunks ---
                        nnorm = len(r["norm_map"])
                        for c0 in range(0, nnorm, _OH_BATCH):
                            cn = min(_OH_BATCH, nnorm - c0)
                            oh = ohpool.tile([P, _OH_BATCH * P], f16,
                                             tag=f"oh{tag}")
                            nc.vector.tensor_tensor(
                                out=oh[:, :cn * P].rearrange(
                                    "p (c d) -> p c d", d=P),
                                in0=dt_[:, c0:c0 + cn].rearrange(
                                    "p (c u) -> p c u", u=1).to_broadcast(
                                    [P, cn, P]),
                                in1=iotarep[:, :cn * P].rearrange(
                                    "p (c d) -> p c d", d=P),
                                op=mybir.AluOpType.is_equal)
                            for cc in range(cn):
                                ci = c0 + cc
                                p2, t_rel = r["norm_map"][ci]
                                s = t_rel // 4
                                mm_seen[s] += 1
                                nc.tensor.matmul(
                                    out=agg[s][:, (t_rel % 4) * P:
                                               (t_rel % 4 + 1) * P],
                                    lhsT=gt[:, ci * P + p2 * HID:
                                            ci * P + p2 * HID + HID],
                                    rhs=oh[:, cc * P:(cc + 1) * P],
                                    start=(mm_seen[s] == 1),
                                    stop=(mm_seen[s] == mm_total[s]),
                                    skip_group_check=True)
                        # --- shared overflow chunks (labels span nt tiles,
                        # one gather chunk, two per-parity dstl columns) ---
                        for k2 in range(r["novf"] // P):
                            ci = nnorm + k2
                            for p2 in (0, 1):
                                if not r["ovf_tiles"][p2]:
                                    continue
                                dcol = nnorm + 2 * k2 + p2
                                oh = ohpool.tile([P, _STRIPE_T[dk] * P], f16,
                                                 tag=f"ohovf{tag}")
                                nc.vector.tensor_tensor(
                                    out=oh[:, :nt * P],
                                    in0=dt_[:, dcol:dcol + 1].to_broadcast(
                                        [P, nt * P]),
                                    in1=iotaT[dk][:, :nt * P],
                                    op=mybir.AluOpType.is_equal)
                                for t_rel in r["ovf_tiles"][p2]:
                                    s = t_rel // 4
                                    mm_seen[s] += 1
                                    nc.tensor.matmul(
                                        out=agg[s][:, (t_rel % 4) * P:
                                                   (t_rel % 4 + 1) * P],
                                        lhsT=gt[:, ci * P + p2 * HID:
                                                ci * P + p2 * HID + HID],
                                        rhs=oh[:, t_rel * P:(t_rel + 1) * P],
                                        start=(mm_seen[s] == 1),
                                        stop=(mm_seen[s] == mm_total[s]),
                                        skip_group_check=True)
                    assert mm_seen == mm_total, (dk, l, si, et, mm_seen, mm_total)
                # --- epilogue: aggs -> z -> relu -> stores ---
                aggsb = {}
                for et in ets:
                    a = pool.tile([HID, _STRIPE_T[dk] * P], f16, tag=f"aggsb{et}",
                                  name=f"aggsb{et}")
                    for s in range(nsub):
                        w = min(4 * P, (nt - s * 4) * P)
                        nc.scalar.activation(
                            out=a[:, s * 4 * P:s * 4 * P + w],
                            in_=aggs[et][s][:, :w],
                            func=mybir.ActivationFunctionType.Copy)
                    aggsb[et] = a
                hTs = pool.tile([HID, _STRIPE_T[dk] * P], f16, tag="hTs")
                nc.sync.dma_start(hTs[:, :nt * P],
                                  hT_loc[dk][l][:][:, ts * P:(ts + nt) * P])
                zsb = pool.tile([P, _STRIPE_T[dk] * HID], f16, tag="zsb")
                for j in range(nt):
                    t = ts + j
                    zmr = psum.tile([P, 4 * HID], f32, space="PSUM", tag="zmr")
                    for ei_, et in enumerate(ets):
                        nc.tensor.matmul(
                            out=zmr[:, ei_ * HID:(ei_ + 1) * HID],
                            lhsT=aggsb[et][:, j * P:(j + 1) * P],
                            rhs=wl_sb[(et, l)][:],
                            start=(ei_ == 0), stop=False,
                            skip_group_check=True)
                    ro = 2 * HID
                    nc.tensor.matmul(out=zmr[:, ro:ro + HID],
                                     lhsT=hTs[:, j * P:(j + 1) * P],
                                     rhs=wrc_sb[(dk, l)][:],
                                     start=False,
                                     stop=not bias_nonzero[dk],
                                     skip_group_check=True)
                    if bias_nonzero[dk]:
                        nc.tensor.matmul(out=zmr[:, ro:ro + HID],
                                         lhsT=ones1[:],
                                         rhs=bc_sb[(dk, l)][:],
                                         start=False, stop=True,
                                         skip_group_check=True)
                    # z = sum_et ivc_et * zm_et + zroot, then relu (fp16 out)
                    zrt = pool.tile([P, HID], f32, tag="zrt")
                    nc.scalar.activation(
                        out=zrt[:], in_=zmr[:, ro:ro + HID],
                        func=mybir.ActivationFunctionType.Copy)
                    tmp = pool.tile([P, HID], f32, tag="ztmp")
                    nc.vector.scalar_tensor_tensor(
                        out=tmp[:],
                        in0=zmr[:, 0:HID],
                        scalar=ivc_sb[ets[0]][:, t:t + 1],
                        in1=zrt[:],
                        op0=mybir.AluOpType.mult,
                        op1=mybir.AluOpType.add)
                    if len(ets) > 1:
                        nc.vector.scalar_tensor_tensor(
                            out=tmp[:],
                            in0=zmr[:, HID:2 * HID],
                            scalar=ivc_sb[ets[1]][:, t:t + 1],
                            in1=tmp[:],
                            op0=mybir.AluOpType.mult,
                            op1=mybir.AluOpType.add)
                    nc.scalar.activation(
                        out=zsb[:, j * HID:(j + 1) * HID], in_=tmp[:],
                        func=mybir.ActivationFunctionType.Relu)
                    if not last:
                        # transposed copy for next layer's root term
                        ztp = psum.tile([HID, P], f16, space="PSUM", tag="ztp")
                        nc.tensor.transpose(
                            out=ztp[:, :],
                            in_=zsb[:, j * HID:(j + 1) * HID],
                            identity=ident[:])
                        ztr2 = pool.tile([HID, P], f16, tag="ztr2")
                        nc.scalar.activation(
                            out=ztr2[:], in_=ztp[:, :],
                            func=mybir.ActivationFunctionType.Copy)
                        nc.sync.dma_start(
                            hT_loc[dk][l + 1][:][:, t * P:(t + 1) * P],
                            ztr2[:])
                if not last:
                    if not (dk == "c" and l == NLAYERS - 2):
                        oap = shard_buf[dk][l][:][ts * P:(ts + nt) * P, :].rearrange(
                            "(t p) f -> p t f", p=P)
                        nc.sync.dma_start(oap, zsb[:, :nt * HID].rearrange(
                            "p (t f) -> p t f", f=HID))
                else:
                    tmp2 = pool.tile([P, TC * HID], f16, tag="ctmp")
                    nc.vector.tensor_tensor(
                        out=tmp2[:, :nt * HID], in0=zsb[:, :nt * HID],
                        in1=wclsr2[:, :nt * HID],
                        op=mybir.AluOpType.mult)
                    ot = pool.tile([P, TC], f32, tag="otile")
                    nc.vector.tensor_reduce(
                        out=ot[:, :nt],
                        in_=tmp2[:, :nt * HID].rearrange(
                            "p (t f) -> p t f", f=HID),
                        axis=mybir.AxisListType.X,
                        op=mybir.AluOpType.add)
                    if b_cls != 0.0:
                        nc.vector.tensor_scalar(
                            out=ot[:, :nt], in0=ot[:, :nt],
                            scalar1=b_cls, scalar2=None,
                            op0=mybir.AluOpType.add)
                    oap = out_t[:].rearrange("(t p) o -> p t o", p=P)
                    nc.sync.dma_start(oap[:, ts:ts + nt, 0], ot[:, :nt])

            # AllGathers are chunked into ~10MB pieces. Pieces whose producer
            # stripes finished ≥2 stripes ago are issued IN-phase (their row
            # slice is already stored, so the in-order Pool queue does not
            # stall); the last piece is deferred into the next dst phase so
            # the collective never head-blocks the next phase's first gathers
            # behind this phase's epilogue tail.
            pending_ag = []  # [(table_key, closure)]

            def ag_piece(dk, l, r0, r1, gbase):
                def _ag():
                    nc.gpsimd.collective_compute(
                        "AllGather", mybir.AluOpType.bypass,
                        ins=[shard_buf[dk][l][:][r0:r1, :]],
                        outs=[ag_out[dk][l][:][
                            gbase:gbase + N_CORES * (r1 - r0), :]],
                        replica_groups=[list(range(N_CORES))])
                return _ag

            def require_tables(dk, l):
                # force-issue any still-pending AllGather pieces whose output
                # tables this phase's gathers read (safety: Tile does NOT
                # reorder a later collective before an earlier read)
                if l == 0:
                    return
                need = {(_ETYPES[et][1], l - 1) for et in _DST_ETYPES[dk]}
                i = 0
                while i < len(pending_ag):
                    key, fn = pending_ag[i]
                    if key in need:
                        fn()
                        pending_ag.pop(i)
                    else:
                        i += 1

            def do_dst(dk, l, interleave=None, per_stripe=0):
                last = (l == NLAYERS - 1)
                require_tables(dk, l)
                stripes = prep[_DST_ETYPES[dk][0]]["stripes"]
                nstr = len(stripes)
                own_pieces = {}
                deferred = []
                if not last and not (dk == "c" and l == NLAYERS - 2):
                    gbase = 0
                    for (r0, r1, es) in pieces[dk]:
                        piece = ag_piece(dk, l, r0, r1, gbase)
                        gbase += N_CORES * (r1 - r0)
                        trig = es + 1  # 2-stripe lag behind producer
                        if trig <= nstr - 2:
                            own_pieces.setdefault(trig, []).append(piece)
                        else:
                            deferred.append(((dk, l), piece))
                for si in range(nstr):
                    do_stripe(dk, l, si, last)
                    if pending_ag and si >= 1:
                        pending_ag.pop(0)[1]()
                    for fn in own_pieces.pop(si, []):
                        fn()
                    if interleave is not None:
                        for _ in range(per_stripe):
                            if next(interleave, "done") == "done":
                                interleave = None
                                break
                if interleave is not None:
                    for _ in interleave:
                        pass
                pending_ag.extend(deferred)

            order = {0: ["c", "r", "j"], 1: ["r", "j", "c"], 2: ["c"]}
            for l in range(NLAYERS):
                for dk in order[min(l, 2)] if NLAYERS == 3 else ["c", "r", "j"]:
                    if l == NLAYERS - 1 and dk != "c":
                        continue
                    if l == 0 and dk == "c":
                        # cheval projection interleaved among the c-dst l0
                        # stripes so h0_c is ready when r-dst l0 starts
                        nstr = len(prep[_DST_ETYPES["c"][0]]["stripes"])
                        ngroups = _ceil(npad["c"] // P, GB)
                        do_dst(dk, l, interleave=proj_full_gen("c"),
                               per_stripe=_ceil(ngroups, nstr))
                    else:
                        do_dst(dk, l)
            for _, fn in pending_ag:
                fn()
            pending_ag.clear()

    nc.finalize()

    iota128_v = np.broadcast_to(np.arange(P, dtype=np.float16), (P, P)).copy()
    iotarep_v = np.broadcast_to(
        np.tile(np.arange(P, dtype=np.float16), _OH_BATCH),
        (P, _OH_BATCH * P)).copy()
    iotaT_v = {dk: np.broadcast_to(
        np.arange(_STRIPE_T[dk] * P, dtype=np.float16),
        (P, _STRIPE_T[dk] * P)).copy() for dk in _DST_ETYPES}
    wclsr_v = np.tile(w_cls.reshape(1, HID), (P, TC)).astype(np.float16)

    in_maps = []
    for c in range(N_CORES):
        m = {}
        for k in tpc:
            sh = tpc[k] * P
            m[f"xT_{k}"] = xT[k]
            m[f"xo_{k}"] = np.ascontiguousarray(xT[k][:, c * sh:(c + 1) * sh])
            m[f"win_{k}"] = w_in_np[k]
        for et in _ETYPES:
            m[f"idx_{et}"] = prep[et]["idx"][c]
            m[f"dstl_{et}"] = prep[et]["dstl"][c]
            m[f"ivc_{et}"] = np.ascontiguousarray(iv[et][c].T)
            m[f"wl_{et}"] = WL[et]
        for dk in _DST_ETYPES:
            m[f"wrc_{dk}"] = WRc[dk]
            m[f"bc_{dk}"] = Bc[dk].reshape(NLAYERS, 1, HID)
            m[f"iotaT_{dk}"] = iotaT_v[dk]
        m["iota128"] = iota128_v
        m["iotarep"] = iotarep_v
        m["wclsr"] = wclsr_v
        in_maps.append(m)

    install_neuronx_cc_hook()
    partition_name = nc.partition_id_tensor.name if nc.partition_id_tensor else None
    in_names, out_names, out_avals, zero_outs = [], [], [], []
    for alloc in nc.m.functions[0].allocations:
        if not isinstance(alloc, mybir.MemoryLocationSet):
            continue
        name = alloc.memorylocations[0].name
        if alloc.kind == "ExternalInput":
            if name != partition_name:
                in_names.append(name)
        elif alloc.kind == "ExternalOutput":
            out_names.append(name)
            shape = tuple(alloc.tensor_shape)
            dtype = mybir.dt.np(alloc.dtype)
            out_avals.append(jax.core.ShapedArray(shape, dtype))
            zero_outs.append(np.zeros(shape, dtype))
    n_params = len(in_names)
    all_in = list(in_names) + list(out_names)
    if partition_name is not None:
        all_in.append(partition_name)

    def _body(*args):
        operands = list(args)
        if partition_name is not None:
            operands.append(partition_id_tensor())
        outs = _bass_exec_p.bind(
            *operands, out_avals=tuple(out_avals), in_names=tuple(all_in),
            out_names=tuple(out_names), lowering_input_output_aliases=(),
            sim_require_finite=False, sim_require_nnan=False, nc=nc)
        return tuple(outs)

    devices = jax.devices()[:N_CORES]
    mesh = Mesh(np.asarray(devices), ("core",))
    specs = (PartitionSpec("core"),)
    sharded = jax.jit(
        shard_map(_body, mesh=mesh, in_specs=specs * (n_params + len(out_names)),
                  out_specs=specs * len(out_names), check_rep=False),
        keep_unused=True)
    per_core = [[np.asarray(m[n]) for n in in_names] for m in in_maps]
    concat_in = [np.concatenate([per_core[c][i] for c in range(N_CORES)], axis=0)
                 for i in range(n_params)]
    concat_zero = [np.zeros((N_CORES * z.shape[0], *z.shape[1:]), z.dtype)
                   for z in zero_outs]
    shd = NamedSharding(mesh, PartitionSpec("core"))
    dev_in = [
        jax.make_array_from_callback(a.shape, shd, lambda idx, a=a: a[idx])
        for a in concat_in + concat_zero
    ]
    outs = sharded(*dev_in)
    jax.block_until_ready(outs)
    global _PROF
    _PROF = dict(sharded=sharded, dev_in=dev_in, nc=nc, out_names=out_names)
    oi = out_names.index("out")
    full = np.asarray(outs[oi]).reshape(N_CORES * tpc["c"] * P, 1)
    return full[:NC, :].astype(np.float32)



# revision 22
# speedup vs baseline: 1.4143x; 1.0085x over previous
"""Trainium2 Bass kernel for 3-layer heterogeneous GraphSAGE (EntityGraphNN).

8 NeuronCores, SPMD single program, fp16 data path (f32 PSUM accumulate):
  - Destination-node sharding: each core owns 1/8 of each node type's
    128-row tiles. Edges routed to the core owning their dst.
  - h tables stored fp16 [N, 64]; gathers fetch PAIRS of rows (256B) via
    gpsimd dma_gather. Pair-index space is banked by 32768 (int16 limit);
    cells are (tile, pairbank, src-parity) so every 128-slot chunk is
    parity-pure and the matmul lhsT picks the right 64-wide half.
  - Scatter-add via one-hot matmul: oh[e,d] = (dstl[e]==d) built on DVE in
    BATCHED 3-D broadcast ops (one instr per <=16 chunks), fp16; PSUM
    agg_T[64, 512] accumulates G^T @ oh per 4-tile sub-stripe.
  - z = (agg@Wl)*invcnt + h_dst@Wr (+b); ReLU; store h fp16 row-major
    (shard -> AllGather table) and transposed (next layer's root term).
  - Input projections replicated on every core (no layer-0 AllGather).
  - AllGather per dst type right after its stripes finish; layer order
    c,r,j then r,j,c then c, so each AG overlaps the following compute.
    AG of cheval after layer 1 is never consumed -> skipped.
  - No global barriers: Tile's shadow-memory dep tracking orders the
    collectives against producers/consumers.
"""
import numpy as np

HID = 64
P = 128
PBANK = 32768  # pairs per gather bank (int16 index range)
N_CORES = 8
OVF_CAP = 1024  # max shared overflow slots per run before q escalation

_ETYPES = {
    "rev_part": ("c", "r"),
    "monte": ("c", "j"),
    "part": ("r", "c"),
    "rev_monte": ("j", "c"),
}
_DST_ETYPES = {"c": ["rev_part", "monte"], "r": ["part"], "j": ["rev_monte"]}
_STRIPE_T = {"c": 8, "r": 16, "j": 8}
_OH_BATCH = 16  # chunks per batched one-hot DVE op


def _ceil(a, b):
    return (a + b - 1) // b


def _prep_edge_type(ei, n_src_pad, tpc, q, T):
    """Pack edges into (stripe, pairbank) gather runs with parity-pure
    128-slot chunks. Returns None if any (run, parity) overflow exceeds
    OVF_CAP (caller escalates q)."""
    src = ei[0].astype(np.int64)
    dst = ei[1].astype(np.int64)
    npair = n_src_pad // 2
    nbp = _ceil(npair, PBANK)
    shard = tpc * P
    core = dst // shard
    tile = (dst % shard) // P
    pair = src >> 1
    par = src & 1
    pb = pair // PBANK

    stripes = []
    t0 = 0
    while t0 < tpc:
        stripes.append((t0, min(T, tpc - t0)))
        t0 += T

    order = np.lexsort((src, par, pb, tile, core))
    so, do_ = src[order], dst[order]
    key = ((core * tpc + tile) * nbp + pb) * 2 + par
    ko = key[order]
    ncell = N_CORES * tpc * nbp * 2
    cnts = np.bincount(key, minlength=ncell).reshape(N_CORES, tpc, nbp, 2)
    starts = np.zeros(ncell + 1, np.int64)
    np.cumsum(cnts.reshape(-1), out=starts[1:])

    cpt = q // P
    runs = []
    total_slots = 0
    total_dcols = 0
    for si, (ts, nt) in enumerate(stripes):
        for b in range(nbp):
            # shared (both-parity) overflow block, 128-padded
            oc_max = 0
            ovf_tiles = [set(), set()]
            for c in range(N_CORES):
                oc = 0
                for p2 in (0, 1):
                    ov = np.maximum(cnts[c, ts:ts + nt, b, p2] - q, 0)
                    oc += int(ov.sum())
                    for t_rel in np.nonzero(ov)[0]:
                        ovf_tiles[p2].add(int(t_rel))
                oc_max = max(oc_max, oc)
            if oc_max > OVF_CAP:
                return None
            novf = _ceil(oc_max, P) * P
            n_slots = 2 * nt * q + novf
            norm_map = [(p2, t_rel)
                        for p2 in (0, 1)
                        for t_rel in range(nt)
                        for _ in range(cpt)]
            runs.append(dict(si=si, b=b, ts=ts, nt=nt, n_slots=n_slots,
                             novf=novf,
                             ovf_tiles=(sorted(ovf_tiles[0]),
                                        sorted(ovf_tiles[1])),
                             norm_map=norm_map))
            total_slots += n_slots
            # dstl columns: 1 per normal chunk, 2 per shared overflow chunk
            total_dcols += len(norm_map) + 2 * (novf // P)

    w_tot = total_slots // 16
    idx_all = np.zeros((N_CORES, 16, w_tot), np.int16)
    dstl_all = np.full((N_CORES, P, total_dcols), -1.0, np.float16)

    wofs = 0
    cofs = 0
    for r in runs:
        ts, nt, b = r["ts"], r["nt"], r["b"]
        n_slots = r["n_slots"]
        novf = r["novf"]
        nnorm = len(r["norm_map"])
        for c in range(N_CORES):
            idxs = np.zeros(n_slots, np.int64)
            # normal-chunk labels + per-parity overflow labels
            dls = np.full((n_slots // P, P), -1.0, np.float32)
            ovf_lab = [np.full(novf, -1.0, np.float32) for _ in (0, 1)]
            base_ovf = 2 * nt * q
            ovf_off = 0
            for p2 in (0, 1):
                for t_rel in range(nt):
                    t = ts + t_rel
                    k = ((c * tpc + t) * nbp + b) * 2 + p2
                    s0, s1 = int(starts[k]), int(starts[k + 1])
                    cell_pair = (so[s0:s1] >> 1) - b * PBANK
                    cell_dst = do_[s0:s1] - (c * shard + t * P)
                    take = min(s1 - s0, q)
                    blk = (p2 * nt + t_rel) * q
                    idxs[blk:blk + take] = cell_pair[:take]
                    dls.reshape(-1)[blk:blk + take] = cell_dst[:take]
                    if s1 - s0 > take:
                        no = s1 - s0 - take
                        assert ovf_off + no <= novf
                        idxs[base_ovf + ovf_off:base_ovf + ovf_off + no] = \
                            cell_pair[take:]
                        ovf_lab[p2][ovf_off:ovf_off + no] = \
                            cell_dst[take:] + t_rel * P
                        ovf_off += no
            idx_all[c, :, wofs:wofs + n_slots // 16] = \
                idxs.reshape(n_slots // 16, 16).T
            # dstl layout: [normal chunks][ovf chunk p0, ovf chunk p1]...
            dcols = np.full((total_dcols_run(r), P), -1.0, np.float32)
            dcols[:nnorm] = dls[:nnorm]
            for k2 in range(novf // P):
                dcols[nnorm + 2 * k2] = ovf_lab[0][k2 * P:(k2 + 1) * P]
                dcols[nnorm + 2 * k2 + 1] = ovf_lab[1][k2 * P:(k2 + 1) * P]
            dstl_all[c, :, cofs:cofs + dcols.shape[0]] = \
                dcols.T.astype(np.float16)
        r["wofs"] = wofs
        r["cofs"] = cofs
        r["ch"] = n_slots // P
        r["dch"] = total_dcols_run(r)
        wofs += n_slots // 16
        cofs += r["dch"]

    idx_rep = np.tile(idx_all, (1, 8, 1))
    return dict(runs=runs, stripes=stripes, idx=idx_rep, dstl=dstl_all,
                nbp=nbp, q=q, cpt=cpt, n_chunks=total_dcols, w_tot=w_tot,
                max_ch=max(r["ch"] for r in runs),
                max_dch=max(r["dch"] for r in runs))


def total_dcols_run(r):
    return len(r["norm_map"]) + 2 * (r["novf"] // P)


def _prep_type(ei, n_src_pad, tpc, T):
    nbp = _ceil(n_src_pad // 2, PBANK)
    lam_cell = ei.shape[1] / (N_CORES * tpc) / (nbp * 2)
    q = max(P, (int(lam_cell) // P) * P)
    while True:
        r = _prep_edge_type(ei, n_src_pad, tpc, q, T)
        if r is not None:
            return r
        q += P


def _invcnt(ei, n_dst_pad, tpc):
    cnt = np.bincount(ei[1].astype(np.int64), minlength=n_dst_pad).astype(np.float32)
    inv = 1.0 / np.maximum(cnt, 1.0)
    return inv.reshape(N_CORES, tpc, P)


def kernel(**inputs):
    import concourse.bass as bass
    import concourse.mybir as mybir
    import concourse.tile as tile
    import concourse.bacc as bacc
    import jax
    from jax.sharding import Mesh, PartitionSpec, NamedSharding
    from jax.experimental.shard_map import shard_map
    from concourse.bass2jax import (_bass_exec_p, partition_id_tensor,
                                    install_neuronx_cc_hook)

    f32 = mybir.dt.float32
    f16 = mybir.dt.float16
    x_np = {"c": np.asarray(inputs["x_cheval"], np.float32),
            "j": np.asarray(inputs["x_jockey"], np.float32),
            "r": np.asarray(inputs["x_course"], np.float32)}
    NC = x_np["c"].shape[0]

    tpc = {k: _ceil(x_np[k].shape[0], P * N_CORES) for k in x_np}
    npad = {k: tpc[k] * P * N_CORES for k in tpc}
    din = {k: x_np[k].shape[1] + 1 for k in x_np}  # +1 ones row folds bias

    xT = {}
    for k in x_np:
        xt = np.zeros((din[k], npad[k]), np.float16)
        xt[:-1, :x_np[k].shape[0]] = x_np[k].T.astype(np.float16)
        xt[-1, :] = 1.0
        xT[k] = xt

    w_in_np = {}
    for k, nm in (("c", "cheval"), ("j", "jockey"), ("r", "course")):
        w = np.asarray(inputs[f"w_in_{nm}"], np.float32)
        b = np.asarray(inputs[f"b_in_{nm}"], np.float32)
        w_in_np[k] = np.concatenate([w, b.reshape(1, HID)], axis=0).astype(np.float16)

    w_cls = np.asarray(inputs["w_cls"], np.float32)
    b_cls = float(np.asarray(inputs["b_cls"]).reshape(-1)[0])
    eis = {k: np.asarray(inputs["ei_" + k]) for k in _ETYPES}
    NLAYERS = np.asarray(inputs["wl_part"]).shape[0]

    # ---- piece-major table layout -------------------------------------
    # AllGathers are chunked; the BIR verifier requires contiguous
    # collective outputs, so tables are stored piece-major: for each row
    # piece [r0:r1) (aligned to stripe boundaries of the producing dst
    # phase), all 8 cores' rows are contiguous. Gather indices are
    # remapped host-side; dst-side structures stay in natural order.
    _N_PIECES = {"c": 4, "r": 2, "j": 1}

    def _mk_stripes(tpc_k, T):
        out = []
        t0 = 0
        while t0 < tpc_k:
            out.append((t0, min(T, tpc_k - t0)))
            t0 += T
        return out

    pieces = {}
    piecemap = {}
    for k in tpc:
        stripes_k = _mk_stripes(tpc[k], _STRIPE_T[k])
        nstr = len(stripes_k)
        npc = min(_N_PIECES[k], nstr)
        bounds = []
        r0 = 0
        for j in range(npc):
            es = ((j + 1) * nstr) // npc
            r1 = (stripes_k[es - 1][0] + stripes_k[es - 1][1]) * P
            if r1 > r0:
                bounds.append((r0, r1, es))
            r0 = r1
        pieces[k] = bounds
        S = tpc[k] * P
        pm = np.empty(npad[k], np.int64)
        base = 0
        for (r0, r1, _es) in bounds:
            sz = r1 - r0
            for c in range(N_CORES):
                pm[c * S + r0:c * S + r1] = base + c * sz + np.arange(sz)
            base += N_CORES * sz
        piecemap[k] = pm

    prep, iv = {}, {}
    for et, (dk, sk) in _ETYPES.items():
        ei_remap = np.stack([piecemap[sk][eis[et][0].astype(np.int64)],
                             eis[et][1].astype(np.int64)])
        prep[et] = _prep_type(ei_remap, npad[sk], tpc[dk], _STRIPE_T[dk])
        iv[et] = _invcnt(eis[et], npad[dk], tpc[dk])

    WL = {et: np.asarray(inputs["wl_" + et], np.float16) for et in _ETYPES}
    BL = {et: np.asarray(inputs["bl_" + et], np.float32) for et in _ETYPES}
    WR = {et: np.asarray(inputs["wr_" + et], np.float32) for et in _ETYPES}
    WRc = {dk: sum(WR[et] for et in _DST_ETYPES[dk]).astype(np.float16)
           for dk in _DST_ETYPES}
    Bc = {dk: sum(BL[et] for et in _DST_ETYPES[dk]).astype(np.float16)
          for dk in _DST_ETYPES}
    bias_nonzero = {dk: bool(np.any(Bc[dk])) for dk in _DST_ETYPES}

    nc = bacc.Bacc(None, num_swdge_queues=4, dynamic_dma_scratch_size=49152)

    din_t = {k: nc.declare_dram_parameter(f"xT_{k}", [din[k], npad[k]], f16, False)
             for k in tpc}
    xown_t = {k: nc.declare_dram_parameter(f"xo_{k}", [din[k], tpc[k] * P], f16, False)
              for k in tpc}
    win_t = {k: nc.declare_dram_parameter(f"win_{k}", [din[k], HID], f16, False)
             for k in tpc}
    idx_t = {et: nc.declare_dram_parameter(f"idx_{et}", [P, prep[et]["w_tot"]],
                                           mybir.dt.int16, False) for et in _ETYPES}
    dstl_t = {et: nc.declare_dram_parameter(f"dstl_{et}", [P, prep[et]["n_chunks"]],
                                            f16, False) for et in _ETYPES}
    ivc_t = {et: nc.declare_dram_parameter(f"ivc_{et}", [P, tpc[_ETYPES[et][0]]],
                                           f32, False) for et in _ETYPES}
    wl_t = {et: nc.declare_dram_parameter(f"wl_{et}", [NLAYERS, HID, HID], f16, False)
            for et in _ETYPES}
    wrc_t = {dk: nc.declare_dram_parameter(f"wrc_{dk}", [NLAYERS, HID, HID], f16,
                                           False) for dk in _DST_ETYPES}
    bc_t = {dk: nc.declare_dram_parameter(f"bc_{dk}", [NLAYERS, 1, HID], f16, False)
            for dk in _DST_ETYPES}
    iota128_t = nc.declare_dram_parameter("iota128", [P, P], f16, False)
    iotarep_t = nc.declare_dram_parameter("iotarep", [P, _OH_BATCH * P], f16, False)
    iotaT_t = {dk: nc.declare_dram_parameter(f"iotaT_{dk}", [P, _STRIPE_T[dk] * P],
                                             f16, False) for dk in _DST_ETYPES}
    wclsr_t = nc.declare_dram_parameter("wclsr", [P, _STRIPE_T["c"] * HID], f16, False)
    out_t = nc.declare_dram_parameter("out", [tpc["c"] * P, 1], f32, True)

    TC = _STRIPE_T["c"]

    with tile.TileContext(nc) as tc:
        with (
            tc.tile_pool(name="wpool", bufs=1) as wpool,
            tc.tile_pool(name="gpool", bufs=3) as gpool,
            tc.tile_pool(name="ohpool", bufs=2) as ohpool,
            tc.tile_pool(name="pool", bufs=2) as pool,
            tc.tile_pool(name="psum", bufs=2, space="PSUM") as psum,
        ):
            h_full = {}
            hT_loc = {}
            shard_buf = {}
            ag_out = {}
            for k in tpc:
                h0 = nc.dram_tensor(f"h0{k}", [npad[k], HID], f16)
                ag_out[k] = [nc.dram_tensor(f"ag{k}{l}", [npad[k], HID], f16,
                                            addr_space="Shared")
                             for l in range(2)]
                h_full[k] = [h0, ag_out[k][0], ag_out[k][1]]
                hT_loc[k] = [nc.dram_tensor(f"hT{k}{l}", [HID, tpc[k] * P], f16)
                             for l in range(3)]
                shard_buf[k] = [nc.dram_tensor(f"sh{k}{l}", [tpc[k] * P, HID], f16)
                                for l in range(2)]

            from concourse.masks import make_identity
            ident = wpool.tile([P, P], f16)
            make_identity(nc, ident[:])
            iota128 = wpool.tile([P, P], f16)
            nc.sync.dma_start(iota128[:], iota128_t[:])
            iotarep = wpool.tile([P, _OH_BATCH * P], f16)
            nc.sync.dma_start(iotarep[:], iotarep_t[:])
            iotaT = {}
            for dk in _DST_ETYPES:
                iotaT[dk] = wpool.tile([P, _STRIPE_T[dk] * P], f16,
                                       tag=f"iotaT{dk}", name=f"iotaT{dk}")
                nc.sync.dma_start(iotaT[dk][:], iotaT_t[dk][:])
            wclsr2 = wpool.tile([P, TC * HID], f16)
            nc.sync.dma_start(wclsr2[:], wclsr_t[:])
            ones1 = wpool.tile([1, P], f16)
            nc.gpsimd.memset(ones1[:], 1.0)
            win_sb = {}
            for k in tpc:
                win_sb[k] = wpool.tile([din[k], HID], f16, tag=f"win{k}", name=f"win{k}")
                nc.sync.dma_start(win_sb[k][:], win_t[k][:])
            wl_sb, wrc_sb, bc_sb = {}, {}, {}
            for et in _ETYPES:
                for l in range(NLAYERS):
                    wl_sb[(et, l)] = wpool.tile([HID, HID], f16, tag=f"wl{et}{l}",
                                                name=f"wl{et}{l}")
                    nc.sync.dma_start(wl_sb[(et, l)][:], wl_t[et][l])
            for dk in _DST_ETYPES:
                for l in range(NLAYERS):
                    wrc_sb[(dk, l)] = wpool.tile([HID, HID], f16, tag=f"wrc{dk}{l}",
                                                 name=f"wrc{dk}{l}")
                    nc.sync.dma_start(wrc_sb[(dk, l)][:], wrc_t[dk][l])
                    if bias_nonzero[dk]:
                        bc_sb[(dk, l)] = wpool.tile([1, HID], f16, tag=f"bc{dk}{l}",
                                                    name=f"bc{dk}{l}")
                        nc.sync.dma_start(bc_sb[(dk, l)][:], bc_t[dk][l])
            ivc_sb = {}
            for et in _ETYPES:
                ivc_sb[et] = wpool.tile([P, tpc[_ETYPES[et][0]]], f32,
                                        tag=f"ivc{et}", name=f"ivc{et}")
                nc.sync.dma_start(ivc_sb[et][:], ivc_t[et][:])

            # ---- input projection (full, replicated) ----
            GB = 4

            def proj_full_gen(k):
                # emit in piece-major position order; h0 is stored in the
                # same piece-major layout the AllGather'd tables use
                S = tpc[k] * P
                runs = []
                base_t = 0
                for (r0, r1, _es) in pieces[k]:
                    sz_t = (r1 - r0) // P
                    for c in range(N_CORES):
                        runs.append((base_t + c * sz_t,
                                     (c * S + r0) // P, sz_t))
                    base_t += N_CORES * sz_t
                PG = 8  # proj group: 8 tiles per load/store
                for pos_t0, nat_t0, nt_ in runs:
                    for g in range(0, nt_, PG):
                        gn = min(PG, nt_ - g)
                        xt = gpool.tile([din[k], PG * P], f16, tag="xt")
                        nc.scalar.dma_start(
                            xt[:, :gn * P],
                            din_t[k][:, (nat_t0 + g) * P:(nat_t0 + g + gn) * P])
                        zp = psum.tile([P, PG * HID], f32, space="PSUM", tag="zmr")
                        for j in range(gn):
                            nc.tensor.matmul(
                                out=zp[:, j * HID:(j + 1) * HID],
                                lhsT=xt[:, j * P:(j + 1) * P],
                                rhs=win_sb[k][:], start=(j == 0),
                                stop=(j == gn - 1),
                                skip_group_check=True)
                        zr = pool.tile([P, PG * HID], f16, tag="zr")
                        nc.scalar.activation(
                            out=zr[:, :gn * HID], in_=zp[:, :gn * HID],
                            func=mybir.ActivationFunctionType.Relu)
                        oap = h_full[k][0][:][
                            (pos_t0 + g) * P:(pos_t0 + g + gn) * P, :].rearrange(
                            "(t p) f -> p t f", p=P)
                        nc.scalar.dma_start(oap, zr[:, :gn * HID].rearrange(
                            "p (t f) -> p t f", f=HID))
                        yield

            def proj_full(k):
                for _ in proj_full_gen(k):
                    pass

            def proj_own(k, GB=4):
                for g0 in range(0, tpc[k], GB):
                    gn = min(GB, tpc[k] - g0)
                    xt = gpool.tile([din[k], 8 * P], f16, tag="xt")
                    nc.scalar.dma_start(xt[:, :gn * P],
                                        xown_t[k][:, g0 * P:(g0 + gn) * P])
                    ztp = psum.tile([HID, GB * P], f32, space="PSUM",
                                    tag=f"agg{(g0 // GB) % 2}", bufs=1)
                    for j in range(gn):
                        nc.tensor.matmul(
                            out=ztp[:, j * P:(j + 1) * P],
                            lhsT=win_sb[k][:],
                            rhs=xt[:, j * P:(j + 1) * P], start=(j == 0),
                            stop=(j == gn - 1), skip_group_check=True)
                    ztr = pool.tile([HID, GB * P], f16, tag="ztr")
                    nc.scalar.activation(
                        out=ztr[:, :gn * P], in_=ztp[:, :gn * P],
                        func=mybir.ActivationFunctionType.Relu)
                    nc.scalar.dma_start(hT_loc[k][0][:, g0 * P:(g0 + gn) * P],
                                        ztr[:, :gn * P])

            # Emit projections in the order the first c-dst stripes consume
            # them: course pair-bank 0 (first ~65536 rows), then jockey, then
            # the rest of course. The big cheval projection is interleaved
            # into the c-dst stripe loop below.
            gen_r = proj_full_gen("r")
            nb0 = min(_ceil(2 * PBANK, P * GB), _ceil(npad["r"] // P, GB))
            for _ in range(nb0):
                next(gen_r, None)
            proj_full("j")
            for _ in gen_r:
                pass
            proj_own("j")
            proj_own("r")
            proj_own("c")

            # max gather-tile widths (chunks) per dst: tagA = primary et,
            # tagB = second et of dst c
            gta_ch = max(prep[et]["max_ch"]
                         for et in ("rev_part", "part", "rev_monte"))
            gtb_ch = prep["monte"]["max_ch"]

            qrot = [0]

            def do_stripe(dk, l, si, last):
                ets = _DST_ETYPES[dk]
                stripes = prep[ets[0]]["stripes"]
                ts, nt = stripes[si]
                nsub = _ceil(nt, 4)
                aggs = {}
                for eti, et in enumerate(ets):
                    pr = prep[et]
                    q = pr["q"]
                    cpt = pr["cpt"]
                    rlist = [r for r in pr["runs"] if r["si"] == si]
                    tag = "A" if eti == 0 else "B"
                    mx_ch = gta_ch if eti == 0 else gtb_ch
                    # per (sub) matmul counting for start/stop flags
                    mm_total = [0] * nsub
                    mm_seen = [0] * nsub
                    for r in rlist:
                        for (p2, t_rel) in r["norm_map"]:
                            mm_total[t_rel // 4] += 1
                        for _ in range(r["novf"] // P):
                            for p2 in (0, 1):
                                for t_rel in r["ovf_tiles"][p2]:
                                    mm_total[t_rel // 4] += 1
                    agg = []
                    for s in range(nsub):
                        atag = (f"agg{eti * 2 + s}" if nsub <= 2
                                else f"agg{s}")
                        agg.append(psum.tile([HID, 4 * P], f32, space="PSUM",
                                             tag=atag, name=atag, bufs=1))
                    aggs[et] = agg
                    for r in rlist:
                        ch = r["ch"]
                        n_slots = r["n_slots"]
                        wofs, cofs = r["wofs"], r["cofs"]
                        it = gpool.tile([P, mx_ch * P // 16],
                                        mybir.dt.int16, tag=f"it{tag}")
                        nc.sync.dma_start(it[:, :n_slots // 16],
                                          idx_t[et][:, wofs:wofs + n_slots // 16])
                        dch = r["dch"]
                        dt_ = gpool.tile([P, pr["max_dch"]], f16, tag=f"dt{tag}")
                        nc.sync.dma_start(dt_[:, :dch],
                                          dstl_t[et][:, cofs:cofs + dch])
                        gt = gpool.tile([P, mx_ch * P], f16, tag=f"gt{tag}")
                        sk = _ETYPES[et][1]
                        tab = h_full[sk][l]
                        b = r["b"]
                        pair_lo = b * PBANK
                        pair_hi = min((b + 1) * PBANK, npad[sk] // 2)
                        in_ap = tab[:][2 * pair_lo:2 * pair_hi, :].rearrange(
                            "(pr two) f -> pr (two f)", two=2)
                        # split into <=1024-slot sub-gathers so each engine's
                        # concatenated (single_packet) stream stays within the
                        # 64-descriptor packet ceiling; concatenation lets the
                        # SDMA m2s pipeline its 256B random reads instead of
                        # serializing one HBM latency per descriptor
                        SUB = 1024
                        for s0 in range(0, n_slots, SUB):
                            s1 = min(s0 + SUB, n_slots)
                            nc.gpsimd.dma_gather(
                                out_ap=gt[:, s0:_ceil(s1, P) * P].rearrange(
                                    "p (c f) -> p c f", f=P),
                                in_ap=in_ap,
                                idxs_ap=it[:, s0 // 16:_ceil(s1, 16)],
                                num_idxs=s1 - s0, num_idxs_reg=s1 - s0,
                                elem_size=P, single_packet=True,
                                queue_num=qrot[0] % 4)
                            qrot[0] += 1
                        # --- batched one-hot + matmuls: normal chunks ---
                        nnorm = len(r["norm_map"])
                        for c0 in range(0, nnorm, _OH_BATCH):
                            cn = min(_OH_BATCH, nnorm - c0)
                            oh = ohpool.tile([P, _OH_BATCH * P], f16,
                                             tag=f"oh{tag}")
                            nc.vector.tensor_tensor(
                                out=oh[:, :cn * P].rearrange(
                                    "p (c d) -> p c d", d=P),
                                in0=dt_[:, c0:c0 + cn].rearrange(
                                    "p (c u) -> p c u", u=1).to_broadcast(
                                    [P, cn, P]),
                                in1=iotarep[:, :cn * P].rearrange(
                                    "p (c d) -> p c d", d=P),
                                op=mybir.AluOpType.is_equal)
                            for cc in range(cn):
                                ci = c0 + cc
                                p2, t_rel = r["norm_map"][ci]
                                s = t_rel // 4
                                mm_seen[s] += 1
                                nc.tensor.matmul(
                                    out=agg[s][:, (t_rel % 4) * P:
                                               (t_rel % 4 + 1) * P],
                                    lhsT=gt[:, ci * P + p2 * HID:
                                            ci * P + p2 * HID + HID],
                                    rhs=oh[:, cc * P:(cc + 1) * P],
                                    start=(mm_seen[s] == 1),
                                    stop=(mm_seen[s] == mm_total[s]),
                                    skip_group_check=True)
                        # --- shared overflow chunks (labels span nt tiles,
                        # one gather chunk, two per-parity dstl columns) ---
                        for k2 in range(r["novf"] // P):
                            ci = nnorm + k2
                            for p2 in (0, 1):
                                if not r["ovf_tiles"][p2]:
                                    continue
                                dcol = nnorm + 2 * k2 + p2
                                oh = ohpool.tile([P, _STRIPE_T[dk] * P], f16,
                                                 tag=f"ohovf{tag}")
                                nc.vector.tensor_tensor(
                                    out=oh[:, :nt * P],
                                    in0=dt_[:, dcol:dcol + 1].to_broadcast(
                                        [P, nt * P]),
                                    in1=iotaT[dk][:, :nt * P],
                                    op=mybir.AluOpType.is_equal)
                                for t_rel in r["ovf_tiles"][p2]:
                                    s = t_rel // 4
                                    mm_seen[s] += 1
                                    nc.tensor.matmul(
                                        out=agg[s][:, (t_rel % 4) * P:
                                                   (t_rel % 4 + 1) * P],
                                        lhsT=gt[:, ci * P + p2 * HID:
                                                ci * P + p2 * HID + HID],
                                        rhs=oh[:, t_rel * P:(t_rel + 1) * P],
                                        start=(mm_seen[s] == 1),
                                        stop=(mm_seen[s] == mm_total[s]),
                                        skip_group_check=True)
                    assert mm_seen == mm_total, (dk, l, si, et, mm_seen, mm_total)
                # --- epilogue: aggs -> z -> relu -> stores ---
                aggsb = {}
                for et in ets:
                    a = pool.tile([HID, _STRIPE_T[dk] * P], f16, tag=f"aggsb{et}",
                                  name=f"aggsb{et}")
                    for s in range(nsub):
                        w = min(4 * P, (nt - s * 4) * P)
                        nc.scalar.activation(
                            out=a[:, s * 4 * P:s * 4 * P + w],
                            in_=aggs[et][s][:, :w],
                            func=mybir.ActivationFunctionType.Copy)
                    aggsb[et] = a
                hTs = pool.tile([HID, _STRIPE_T[dk] * P], f16, tag="hTs")
                nc.sync.dma_start(hTs[:, :nt * P],
                                  hT_loc[dk][l][:][:, ts * P:(ts + nt) * P])
                zsb = pool.tile([P, _STRIPE_T[dk] * HID], f16, tag="zsb")
                for j in range(nt):
                    t = ts + j
                    zmr = psum.tile([P, 4 * HID], f32, space="PSUM", tag="zmr")
                    for ei_, et in enumerate(ets):
                        nc.tensor.matmul(
                            out=zmr[:, ei_ * HID:(ei_ + 1) * HID],
                            lhsT=aggsb[et][:, j * P:(j + 1) * P],
                            rhs=wl_sb[(et, l)][:],
                            start=(ei_ == 0), stop=False,
                            skip_group_check=True)
                    ro = 2 * HID
                    nc.tensor.matmul(out=zmr[:, ro:ro + HID],
                                     lhsT=hTs[:, j * P:(j + 1) * P],
                                     rhs=wrc_sb[(dk, l)][:],
                                     start=False,
                                     stop=not bias_nonzero[dk],
                                     skip_group_check=True)
                    if bias_nonzero[dk]:
                        nc.tensor.matmul(out=zmr[:, ro:ro + HID],
                                         lhsT=ones1[:],
                                         rhs=bc_sb[(dk, l)][:],
                                         start=False, stop=True,
                                         skip_group_check=True)
                    # z = sum_et ivc_et * zm_et + zroot, then relu (fp16 out)
                    zrt = pool.tile([P, HID], f32, tag="zrt")
                    nc.scalar.activation(
                        out=zrt[:], in_=zmr[:, ro:ro + HID],
                        func=mybir.ActivationFunctionType.Copy)
                    tmp = pool.tile([P, HID], f32, tag="ztmp")
                    nc.vector.scalar_tensor_tensor(
                        out=tmp[:],
                        in0=zmr[:, 0:HID],
                        scalar=ivc_sb[ets[0]][:, t:t + 1],
                        in1=zrt[:],
                        op0=mybir.AluOpType.mult,
                        op1=mybir.AluOpType.add)
                    if len(ets) > 1:
                        nc.vector.scalar_tensor_tensor(
                            out=tmp[:],
                            in0=zmr[:, HID:2 * HID],
                            scalar=ivc_sb[ets[1]][:, t:t + 1],
                            in1=tmp[:],
                            op0=mybir.AluOpType.mult,
                            op1=mybir.AluOpType.add)
                    nc.scalar.activation(
                        out=zsb[:, j * HID:(j + 1) * HID], in_=tmp[:],
                        func=mybir.ActivationFunctionType.Relu)
                    if not last:
                        # transposed copy for next layer's root term
                        ztp = psum.tile([HID, P], f16, space="PSUM", tag="ztp")
                        nc.tensor.transpose(
                            out=ztp[:, :],
                            in_=zsb[:, j * HID:(j + 1) * HID],
                            identity=ident[:])
                        ztr2 = pool.tile([HID, P], f16, tag="ztr2")
                        nc.scalar.activation(
                            out=ztr2[:], in_=ztp[:, :],
                            func=mybir.ActivationFunctionType.Copy)
                        nc.sync.dma_start(
                            hT_loc[dk][l + 1][:][:, t * P:(t + 1) * P],
                            ztr2[:])
                if not last:
                    if not (dk == "c" and l == NLAYERS - 2):
                        oap = shard_buf[dk][l][:][ts * P:(ts + nt) * P, :].rearrange(
                            "(t p) f -> p t f", p=P)
                        nc.sync.dma_start(oap, zsb[:, :nt * HID].rearrange(
                            "p (t f) -> p t f", f=HID))
                else:
                    tmp2 = pool.tile([P, TC * HID], f16, tag="ctmp")
                    nc.vector.tensor_tensor(
                        out=tmp2[:, :nt * HID], in0=zsb[:, :nt * HID],
                        in1=wclsr2[:, :nt * HID],
                        op=mybir.AluOpType.mult)
                    ot = pool.tile([P, TC], f32, tag="otile")
                    nc.vector.tensor_reduce(
                        out=ot[:, :nt],
                        in_=tmp2[:, :nt * HID].rearrange(
                            "p (t f) -> p t f", f=HID),
                        axis=mybir.AxisListType.X,
                        op=mybir.AluOpType.add)
                    if b_cls != 0.0:
                        nc.vector.tensor_scalar(
                            out=ot[:, :nt], in0=ot[:, :nt],
                            scalar1=b_cls, scalar2=None,
                            op0=mybir.AluOpType.add)
                    oap = out_t[:].rearrange("(t p) o -> p t o", p=P)
                    nc.sync.dma_start(oap[:, ts:ts + nt, 0], ot[:, :nt])

            # AllGathers are chunked into ~10MB pieces. Pieces whose producer
            # stripes finished ≥2 stripes ago are issued IN-phase (their row
            # slice is already stored, so the in-order Pool queue does not
            # stall); the last piece is deferred into the next dst phase so
            # the collective never head-blocks the next phase's first gathers
            # behind this phase's epilogue tail.
            pending_ag = []  # [(table_key, closure)]

            def ag_piece(dk, l, r0, r1, gbase):
                def _ag():
                    nc.gpsimd.collective_compute(
                        "AllGather", mybir.AluOpType.bypass,
                        ins=[shard_buf[dk][l][:][r0:r1, :]],
                        outs=[ag_out[dk][l][:][
                            gbase:gbase + N_CORES * (r1 - r0), :]],
                        replica_groups=[list(range(N_CORES))])
                return _ag

            def require_tables(dk, l):
                # force-issue any still-pending AllGather pieces whose output
                # tables this phase's gathers read (safety: Tile does NOT
                # reorder a later collective before an earlier read)
                if l == 0:
                    return
                need = {(_ETYPES[et][1], l - 1) for et in _DST_ETYPES[dk]}
                i = 0
                while i < len(pending_ag):
                    key, fn = pending_ag[i]
                    if key in need:
                        fn()
                        pending_ag.pop(i)
                    else:
                        i += 1

            def do_dst(dk, l, interleave=None, per_stripe=0):
                last = (l == NLAYERS - 1)
                require_tables(dk, l)
                stripes = prep[_DST_ETYPES[dk][0]]["stripes"]
                nstr = len(stripes)
                own_pieces = {}
                deferred = []
                if not last and not (dk == "c" and l == NLAYERS - 2):
                    gbase = 0
                    for (r0, r1, es) in pieces[dk]:
                        piece = ag_piece(dk, l, r0, r1, gbase)
                        gbase += N_CORES * (r1 - r0)
                        trig = es + 1  # 2-stripe lag behind producer
                        if trig <= nstr - 2:
                            own_pieces.setdefault(trig, []).append(piece)
                        else:
                            deferred.append(((dk, l), piece))
                for si in range(nstr):
                    do_stripe(dk, l, si, last)
                    if pending_ag and si >= 1:
                        pending_ag.pop(0)[1]()
                    for fn in own_pieces.pop(si, []):
                        fn()
                    if interleave is not None:
                        for _ in range(per_stripe):
                            if next(interleave, "done") == "done":
                                interleave = None
                                break
                if interleave is not None:
                    for _ in interleave:
                        pass
                pending_ag.extend(deferred)

            order = {0: ["c", "r", "j"], 1: ["r", "j", "c"], 2: ["c"]}
            for l in range(NLAYERS):
                for dk in order[min(l, 2)] if NLAYERS == 3 else ["c", "r", "j"]:
                    if l == NLAYERS - 1 and dk != "c":
                        continue
                    if l == 0 and dk == "c":
                        # cheval projection interleaved among the c-dst l0
                        # stripes so h0_c is ready when r-dst l0 starts
                        nstr = len(prep[_DST_ETYPES["c"][0]]["stripes"])
                        ngroups = _ceil(npad["c"] // P, GB)
                        do_dst(dk, l, interleave=proj_full_gen("c"),
                               per_stripe=_ceil(ngroups, nstr))
                    else:
                        do_dst(dk, l)
            for _, fn in pending_ag:
                fn()
            pending_ag.clear()

    nc.finalize()

    iota128_v = np.broadcast_to(np.arange(P, dtype=np.float16), (P, P)).copy()
    iotarep_v = np.broadcast_to(
        np.tile(np.arange(P, dtype=np.float16), _OH_BATCH),
        (P, _OH_BATCH * P)).copy()
    iotaT_v = {dk: np.broadcast_to(
        np.arange(_STRIPE_T[dk] * P, dtype=np.float16),
        (P, _STRIPE_T[dk] * P)).copy() for dk in _DST_ETYPES}
    wclsr_v = np.tile(w_cls.reshape(1, HID), (P, TC)).astype(np.float16)

    in_maps = []
    for c in range(N_CORES):
        m = {}
        for k in tpc:
            sh = tpc[k] * P
            m[f"xT_{k}"] = xT[k]
            m[f"xo_{k}"] = np.ascontiguousarray(xT[k][:, c * sh:(c + 1) * sh])
            m[f"win_{k}"] = w_in_np[k]
        for et in _ETYPES:
            m[f"idx_{et}"] = prep[et]["idx"][c]
            m[f"dstl_{et}"] = prep[et]["dstl"][c]
            m[f"ivc_{et}"] = np.ascontiguousarray(iv[et][c].T)
            m[f"wl_{et}"] = WL[et]
        for dk in _DST_ETYPES:
            m[f"wrc_{dk}"] = WRc[dk]
            m[f"bc_{dk}"] = Bc[dk].reshape(NLAYERS, 1, HID)
            m[f"iotaT_{dk}"] = iotaT_v[dk]
        m["iota128"] = iota128_v
        m["iotarep"] = iotarep_v
        m["wclsr"] = wclsr_v
        in_maps.append(m)

    install_neuronx_cc_hook()
    partition_name = nc.partition_id_tensor.name if nc.partition_id_tensor else None
    in_names, out_names, out_avals, zero_outs = [], [], [], []
    for alloc in nc.m.functions[0].allocations:
        if not isinstance(alloc, mybir.MemoryLocationSet):
            continue
        name = alloc.memorylocations[0].name
        if alloc.kind == "ExternalInput":
            if name != partition_name:
                in_names.append(name)
        elif alloc.kind == "ExternalOutput":
            out_names.append(name)
            shape = tuple(alloc.tensor_shape)
            dtype = mybir.dt.np(alloc.dtype)
            out_avals.append(jax.core.ShapedArray(shape, dtype))
            zero_outs.append(np.zeros(shape, dtype))
    n_params = len(in_names)
    all_in = list(in_names) + list(out_names)
    if partition_name is not None:
        all_in.append(partition_name)

    def _body(*args):
        operands = list(args)
        if partition_name is not None:
            operands.append(partition_id_tensor())
        outs = _bass_exec_p.bind(
            *operands, out_avals=tuple(out_avals), in_names=tuple(all_in),
            out_names=tuple(out_names), lowering_input_output_aliases=(),
            sim_require_finite=False, sim_require_nnan=False, nc=nc)
        return tuple(outs)

    devices = jax.devices()[:N_CORES]
    mesh = Mesh(np.asarray(devices), ("core",))
    specs = (PartitionSpec("core"),)
    sharded = jax.jit(
        shard_map(_body, mesh=mesh, in_specs=specs * (n_params + len(out_names)),
                  out_specs=specs * len(out_names), check_rep=False),
        keep_unused=True)
    per_core = [[np.asarray(m[n]) for n in in_names] for m in in_maps]
    concat_in = [np.concatenate([per_core[c][i] for c in range(N_CORES)], axis=0)
                 for i in range(n_params)]
    concat_zero = [np.zeros((N_CORES * z.shape[0], *z.shape[1:]), z.dtype)
                   for z in zero_outs]
    shd = NamedSharding(mesh, PartitionSpec("core"))
    dev_in = [
        jax.make_array_from_callback(a.shape, shd, lambda idx, a=a: a[idx])
        for a in concat_in + concat_zero
    ]
    outs = sharded(*dev_in)
    jax.block_until_ready(outs)
    global _PROF
    _PROF = dict(sharded=sharded, dev_in=dev_in, nc=nc, out_names=out_names)
    oi = out_names.index("out")
    full = np.asarray(outs[oi]).reshape(N_CORES * tpc["c"] * P, 1)
    return full[:NC, :].astype(np.float32)

